# revision 42
# baseline (speedup 1.0000x reference)
"""Trainium2 Bass kernel for nn_HGNNEncoder (DMPNN + global bond attention).

Sharding: data-parallel over bonds/atoms/molecules across 8 NeuronCores.

Key structure (v2):
 - Weight folding: scores = (M Wq)(M Wk)^T = M Wqk M^T with Wqk = Wq Wk^T,
   and att = (P V) Wa = (P M)(Wv Wa) = (P M) Wva.  So the attention needs
   only the raw message M as both K and V -> no K/V compute at all.
 - One AllGather per iteration boundary carrying [msgN | msgT] in bf16
   (1 MB per rank).  M^T (for QK lhsT) and M-natural+ones (for PV rhs)
   are DMA'd back from the AG output.
 - DMPNN neighbor-sum via one dma_gather(transpose=True) per iteration
   (7*1024 rows, bf16) + DVE tree-sum -> directly transposed T^T.
 - bf16 everywhere on the matmul paths, fp32 PSUM accumulation, exp on
   ScalarE with the 1/16 scale folded in, sigmoid computed as
   1/(1+exp(-x)) to avoid ACT table swaps.
"""

import numpy as np

import concourse.bass as bass
import concourse.bacc as bacc
import concourse.mybir as mybir
import concourse.tile as tile
from concourse.bass_utils import run_bass_kernel_spmd

NC = 8
B, NA, MNB = 8192, 4096, 6
H = 256
F = 133
FD = 147
BS = B // NC          # 1024 bonds per core
AS = NA // NC         # 512 atoms per core
APM = 64              # atoms per molecule
MPC = AS // APM       # 8 molecules per core
NITER = 3
NBI = 7 * BS          # 7168 dmpnn gather idxs
ROI = MNB * AS        # 3072 readout gather idxs
PAYR = 2 * BS         # 2048 rows of 256 per rank in the AG payload

f32 = mybir.dt.float32
f32r = mybir.dt.float32r
bf16 = mybir.dt.bfloat16
i16 = mybir.dt.int16
AF = mybir.ActivationFunctionType
ALU = mybir.AluOpType
AX = mybir.AxisListType

SCALE_BOND = float(1.0 / np.sqrt(np.float32(H)))
SCALE_ATOM = float(1.0 / np.sqrt(np.float32(F)))

_CACHE = {}


def _build():
    nc = bacc.Bacc("TRN2", target_bir_lowering=False, debug=False, num_devices=NC)

    def inp(name, shape, dt=f32):
        return nc.dram_tensor(name, list(shape), dt, kind="ExternalInput")

    fbT = inp("fbT", [H, BS])
    faT = inp("faT", [F, AS])
    fa = inp("fa", [AS, F])
    wq_a = inp("wq_a", [F, F]); wk_a = inp("wk_a", [F, F]); wv_a = inp("wv_a", [F, F])
    ln_g = inp("ln_g", [1, F]); ln_b = inp("ln_b", [1, F])
    wi = inp("wi", [H, H])
    wqk = inp("wqk", [H, H]); wh = inp("wh", [H, H]); wva = inp("wva", [H, H])
    wa = inp("wa", [H, H])
    wal1 = inp("wal1", [H, 1]); wal2 = inp("wal2", [H, 1]); nwalb = inp("nwalb", [1, 1])
    wo_f = inp("wo_f", [F, H]); wo_m = inp("wo_m", [H, H]); wo_b = inp("wo_b", [1, H])
    wb_w = inp("wb_w", [H, H]); wb_b = inp("wb_b", [1, H])
    ident_in = inp("ident", [128, 128])
    onesr_in = inp("onesr", [1, 128])
    onesc_in = inp("onesc", [128, 1])
    nb_idx = inp("nb_idx", [128, 7, BS // 128], mybir.dt.int32)
    ro_idx = inp("ro_idx", [128, MNB, AS // 128], mybir.dt.int32)
    out_mol = nc.dram_tensor("mol_out", [MPC, H], f32, kind="ExternalOutput")

    with tile.TileContext(nc) as tc, \
         tc.tile_pool(name="persist", bufs=1) as per, \
         tc.tile_pool(name="dram", bufs=1, space="DRAM") as dram, \
         tc.tile_pool(name="psA", bufs=2, space="PSUM") as ppsA:

        def load_round(pool, src_ap, shape, name, dt=f32r, raw_pool=None):
            raw = (raw_pool or pool).tile(list(shape), f32, name=name + "_raw")
            nc.sync.dma_start(raw[:], src_ap)
            if dt == f32:
                return raw
            t = pool.tile(list(shape), dt, name=name)
            nc.vector.tensor_copy(t[:], raw[:])
            return t

        # ---------------- persistent weights ----------------
        with tc.tile_pool(name="raws", bufs=1) as raws:
            def loadw(src, name, cols=H, dt=bf16):
                return load_round(per, src[:].rearrange("(t p) h -> p t h", p=128),
                                  [128, 2, cols], name, dt=dt, raw_pool=raws)
            Wi = loadw(wi, "Wi", dt=f32r); Wqk = loadw(wqk, "Wqk", dt=f32r)
            Wh = loadw(wh, "Wh", dt=f32r)
            Wva = loadw(wva, "Wva"); Wom = loadw(wo_m, "Wom", dt=f32r)
            War = loadw(wa, "War", dt=f32r); Wbw = loadw(wb_w, "Wbw", dt=f32r)
            Wal1 = loadw(wal1, "Wal1", cols=1, dt=f32r)
            Wal2 = loadw(wal2, "Wal2", cols=1, dt=f32r)
            Wof_h = load_round(per, wo_f[0:128, :], [128, H], "Wof_h", raw_pool=raws)
            Wof_l = load_round(per, wo_f[128:F, :], [5, H], "Wof_l", raw_pool=raws)
            Wob = load_round(per, wo_b[:], [1, H], "Wob", raw_pool=raws)
            Wbb = load_round(per, wb_b[:], [1, H], "Wbb", raw_pool=raws)
            Ident = load_round(per, ident_in[:], [128, 128], "Ident", dt=f32)
            IdentB = load_round(per, ident_in[:], [128, 128], "IdentB", dt=bf16,
                                raw_pool=raws)
            Onesr = load_round(per, onesr_in[:], [1, 128], "Onesr", raw_pool=raws)
            OnesC = load_round(per, onesc_in[:], [128, 1], "OnesC", raw_pool=raws)
            NWalb = load_round(per, nwalb[:], [1, 1], "NWalb", dt=f32)
            OnesrF = load_round(per, onesr_in[:], [1, 128], "OnesrF", dt=f32)
            NbIdx = per.tile([128, 7, BS // 128], mybir.dt.int32, name="NbIdx")
            nc.sync.dma_start(NbIdx[:], nb_idx[:])
            RoIdx = per.tile([128, MNB, AS // 128], mybir.dt.int32, name="RoIdx")
            nc.sync.dma_start(RoIdx[:], ro_idx[:])

        # persistent activations
        InpT = per.tile([128, 2, BS], f32, name="InpT")
        MsgTx = per.tile([128, 2, BS], bf16, name="MsgTx")
        MsgTxF = per.tile([128, 2, BS], f32r, name="MsgTxF")
        MsgN = per.tile([128, BS // 128, H], bf16, name="MsgN")
        FeT_h = per.tile([128, AS], f32r, name="FeT_h")
        FeT_l = per.tile([5, AS], f32r, name="FeT_l")

        agm_in = [dram.tile([BS, H], bf16, name=f"agm_in{t}")
                  for t in range(NITER + 1)]
        agm_out = [dram.tile([B, H], bf16, name=f"agm_out{t}",
                             addr_space="Shared") for t in range(NITER + 1)]
        RG = [list(range(NC))]

        def boundary(it):
            # natural-layout local message via PE transposes
            for c in range(BS // 128):
                psn = ppsA.tile([128, 512], f32, name="psA")[:].bitcast(bf16)
                for ht in range(2):
                    nc.tensor.transpose(psn[:, bass.ts(ht, 128)],
                                        MsgTx[:, ht, bass.ts(c, 128)], IdentB[:])
                nc.vector.tensor_copy(MsgN[:, c, :], psn[:, 0:H])
            nc.sync.dma_start(
                agm_in[it][:].rearrange("(lb p) e -> p lb e", p=128), MsgN[:])
            nc.gpsimd.collective_compute(
                "AllGather", ALU.bypass, RG,
                ins=[agm_in[it].opt()], outs=[agm_out[it].opt()])

        def qt_prime():
            # Q'^T = Wqk^T @ M^T  (local shard, f32r for precision)
            for ht in range(2):
                for ch in range(2):
                    ps = ppsA.tile([128, 512], f32, name="psA")
                    for kt in range(2):
                        nc.tensor.matmul(
                            ps[:], Wqk[:, kt, bass.ts(ht, 128)],
                            MsgTxF[:, kt, bass.ts(ch, 512)],
                            start=(kt == 0), stop=(kt == 1))
                    nc.vector.tensor_copy(QT[:, ht, bass.ts(ch, 512)], ps[:])

        # ============ Phase B: message_0 = relu(f_bonds @ Wi) ==============
        ns_B = nc.enter_named_scope("B_init", False)[0]
        with tc.tile_pool(name="pb", bufs=1) as pb:
            FbT = load_round(pb, fbT[:].rearrange("(t p) i -> p t i", p=128),
                             [128, 2, BS], "FbT")
            for ht in range(2):
                for ch in range(2):
                    ps = ppsA.tile([128, 512], f32, name="psA")
                    for kt in range(2):
                        nc.tensor.matmul(ps[:], Wi[:, kt, bass.ts(ht, 128)],
                                         FbT[:, kt, bass.ts(ch, 512)],
                                         start=(kt == 0), stop=(kt == 1))
                    nc.vector.tensor_copy(InpT[:, ht, bass.ts(ch, 512)], ps[:])
                    nc.scalar.activation(MsgTx[:, ht, bass.ts(ch, 512)], ps[:],
                                         AF.Relu)
                    nc.scalar.activation(MsgTxF[:, ht, bass.ts(ch, 512)], ps[:],
                                         AF.Relu)
            boundary(0)
        nc.leave_named_scope("B_init", ns_B, False)

        # ============ Phase C: 3 message-passing iterations ================
        with tc.tile_pool(name="psS", bufs=2, space="PSUM") as ppsS, \
             tc.tile_pool(name="psP", bufs=1, space="PSUM") as ppsP, \
             tc.tile_pool(name="pmt", bufs=2) as pmt, \
             tc.tile_pool(name="pcw", bufs=1) as pcw:
            QT = pcw.tile([128, 2, BS], bf16, name="QT")
            MT = pcw.tile([128, 2, NC, BS], bf16, name="MT")
            Mb = pcw.tile([128, B // 128, H + 1], bf16, name="Mb")
            Gn = pcw.tile([128, 7, BS // 128, H], bf16, name="Gn")
            T1 = pcw.tile([128, BS // 128, H], f32, name="T1")
            TT = pcw.tile([128, 2, BS], f32r, name="TT")
            DmT = pcw.tile([128, 2, BS], f32r, name="DmT")
            AmT = pcw.tile([128, 2, BS], f32r, name="AmT")
            AtT = pcw.tile([128, 2, BS], bf16, name="AtT")
            AttA = pcw.tile([128, BS // 128, H], bf16, name="AttA")
            AlB = pcw.tile([128, BS], f32, name="AlB")
            Alp = pcw.tile([1, BS], f32r, name="Alp")
            Ea = pcw.tile([1, BS], f32, name="Ea")
            D1 = pcw.tile([128, BS], f32, name="D1")
            nc.vector.memset(Mb[:], 1.0)   # ones column persists across iters
            qt_prime()
            for it in range(NITER):
                ns_it = nc.enter_named_scope(f"C{it}", False)[0]
                src = agm_out[it]
                # ---- M^T via HW DMA-transpose; M-natural(+ones) plain DMA
                for c in range(NC):
                    for ht in range(2):
                        nc.sync.dma_start(
                            MT[:, ht, c, :],
                            src[c * BS:(c + 1) * BS, bass.ts(ht, 128)],
                            transpose=True)
                for c in range(NC):
                    nc.sync.dma_start(
                        Mb[:, c * 8:(c + 1) * 8, 0:H],
                        src[c * BS:(c + 1) * BS, :]
                        .rearrange("(lb p) e -> p lb e", p=128))
                # ---- dmpnn gather (batched rows); sums/transposes after attn
                ns_g = nc.enter_named_scope(f"C{it}_gather", False)[0]
                for k in range(7):
                    for c in range(BS // 128):
                        nc.gpsimd.indirect_dma_start(
                            out=Gn[:, k, c, :], out_offset=None, in_=src[:],
                            in_offset=bass.IndirectOffsetOnAxis(
                                ap=NbIdx[:, k, c:c + 1], axis=0))
                nc.leave_named_scope(f"C{it}_gather", ns_g, False)

                # ---- attention (rows = shard), flash-style over j blocks
                ns_at = nc.enter_named_scope(f"C{it}_attn", False)[0]
                for ic in range(2):
                    pvp = ppsP.tile([128, 4, 512], f32, name="psP")
                    prev = None
                    for jj in range(B // 128 + 1):
                        if jj < B // 128:
                            c, jl = divmod(jj, 8)
                            sp = ppsS.tile([128, 512], f32, name="psS")
                            for kt in range(2):
                                nc.tensor.matmul(
                                    sp[:], MT[:, kt, c, bass.ts(jl, 128)],
                                    QT[:, kt, bass.ts(ic, 512)],
                                    start=(kt == 0), stop=(kt == 1))
                            pt = pmt.tile([128, 512], bf16, name="PT")
                            nc.scalar.activation(pt[:], sp[:], AF.Exp,
                                                 scale=SCALE_BOND)
                        if prev is not None:
                            pj, ppt = prev
                            for isub in range(4):
                                nc.tensor.matmul(
                                    pvp[:, isub, 0:H + 1],
                                    ppt[:, bass.ts(isub, 128)],
                                    Mb[:, pj, 0:H + 1],
                                    start=(pj == 0), stop=(pj == B // 128 - 1))
                        if jj < B // 128:
                            prev = (jj, pt)
                    for isub in range(4):
                        rec = pmt.tile([128, 1], f32, name="rec")
                        nc.vector.reciprocal(rec[:], pvp[:, isub, H:H + 1])
                        nc.vector.tensor_scalar_mul(
                            AttA[:, ic * 4 + isub, :], pvp[:, isub, 0:H], rec[:])
                nc.leave_named_scope(f"C{it}_attn", ns_at, False)
                ns_po = nc.enter_named_scope(f"C{it}_post", False)[0]

                # ---- tree-sum + transposes (PE queue: after the attention)
                g = lambda k: Gn[:, k, :, :]
                # tree-sum scratch aliases AmT (written later this iteration)
                T2v = AmT[:].rearrange("p t (c e) -> p (t c) e", e=256)
                nc.vector.tensor_tensor(T1[:], g(0), g(1), op=ALU.add)
                nc.vector.tensor_tensor(T2v, g(2), g(3), op=ALU.add)
                nc.vector.tensor_tensor(T1[:], T1[:], T2v, op=ALU.add)
                nc.vector.tensor_tensor(T2v, g(4), g(5), op=ALU.add)
                nc.vector.tensor_tensor(T1[:], T1[:], T2v, op=ALU.add)
                nc.vector.tensor_scalar_mul(T2v, g(6), -1.0)
                nc.vector.tensor_tensor(T1[:], T1[:], T2v, op=ALU.add)
                for c in range(BS // 128):
                    pst = ppsA.tile([128, 512], f32, name="psA")
                    for kt in range(2):
                        nc.tensor.transpose(pst[:, bass.ts(kt, 128)],
                                            T1[:, c, bass.ts(kt, 128)], Ident[:])
                    nc.vector.tensor_copy(
                        TT[:, :, c * 128:(c + 1) * 128],
                        pst[:, 0:256].rearrange("p (t x) -> p t x", t=2))

                # ---- dmpnn matmul (PE, after attention in queue order)
                for ht in range(2):
                    for ch in range(2):
                        ps = ppsA.tile([128, 512], f32, name="psA")
                        for kt in range(2):
                            nc.tensor.matmul(ps[:], Wh[:, kt, bass.ts(ht, 128)],
                                             TT[:, kt, bass.ts(ch, 512)],
                                             start=(kt == 0), stop=(kt == 1))
                        nc.vector.tensor_copy(DmT[:, ht, bass.ts(ch, 512)], ps[:])

                # ---- att^T then att_msg^T = Wva^T @ att^T
                for ht in range(2):
                    for half in range(2):
                        pst = ppsA.tile([128, 512], f32,
                                        name="psA")[:].bitcast(bf16)
                        for cc in range(4):
                            nc.tensor.transpose(
                                pst[:, bass.ts(cc, 128)],
                                AttA[:, half * 4 + cc, bass.ts(ht, 128)],
                                IdentB[:])
                        nc.vector.tensor_copy(AtT[:, ht, bass.ts(half, 512)],
                                              pst[:, 0:512])
                for ht in range(2):
                    for ch in range(2):
                        ps = ppsA.tile([128, 512], f32, name="psA")
                        for kt in range(2):
                            nc.tensor.matmul(ps[:], Wva[:, kt, bass.ts(ht, 128)],
                                             AtT[:, kt, bass.ts(ch, 512)],
                                             start=(kt == 0), stop=(kt == 1))
                        nc.vector.tensor_copy(AmT[:, ht, bass.ts(ch, 512)], ps[:])

                # ---- alpha = sigmoid(w1.dm + w2.am + b) via exp on ScalarE
                for ch in range(2):
                    ps = ppsA.tile([1, 512], f32, name="psA")
                    nc.tensor.matmul(ps[:], Wal1[:, 0, :],
                                     DmT[:, 0, bass.ts(ch, 512)],
                                     start=True, stop=False)
                    nc.tensor.matmul(ps[:], Wal1[:, 1, :],
                                     DmT[:, 1, bass.ts(ch, 512)],
                                     start=False, stop=False)
                    nc.tensor.matmul(ps[:], Wal2[:, 0, :],
                                     AmT[:, 0, bass.ts(ch, 512)],
                                     start=False, stop=False)
                    nc.tensor.matmul(ps[:], Wal2[:, 1, :],
                                     AmT[:, 1, bass.ts(ch, 512)],
                                     start=False, stop=True)
                    nc.scalar.activation(Ea[:, bass.ts(ch, 512)], ps[:],
                                         AF.Exp, scale=-1.0, bias=NWalb[:])
                nc.vector.tensor_scalar_add(Ea[:], Ea[:], 1.0)
                with nc.allow_low_precision(reason="f32r alpha"):
                    nc.vector.reciprocal(Alp[:], Ea[:])
                for ch in range(2):
                    ps = ppsA.tile([128, 512], f32, name="psA")
                    nc.tensor.matmul(ps[:], Onesr[:],
                                     Alp[:, bass.ts(ch, 512)],
                                     start=True, stop=True)
                    nc.vector.tensor_copy(AlB[:, bass.ts(ch, 512)], ps[:])

                # ---- combine (f32)
                for ht in range(2):
                    nc.vector.tensor_tensor(D1[:], DmT[:, ht, :],
                                            AmT[:, ht, :], op=ALU.subtract)
                    nc.vector.tensor_tensor(D1[:], D1[:], AlB[:], op=ALU.mult)
                    nc.vector.tensor_tensor(D1[:], D1[:], AmT[:, ht, :],
                                            op=ALU.add)
                    nc.vector.tensor_tensor(D1[:], D1[:], InpT[:, ht, :],
                                            op=ALU.add)
                    nc.scalar.activation(MsgTx[:, ht, :], D1[:], AF.Relu)
                    nc.scalar.activation(MsgTxF[:, ht, :], D1[:], AF.Relu)
                boundary(it + 1)
                if it + 1 < NITER:
                    qt_prime()
                nc.leave_named_scope(f"C{it}_post", ns_po, False)
                nc.leave_named_scope(f"C{it}", ns_it, False)

        # ============ Phase A: per-molecule atom self-attention ============
        # (emitted late: fills the final-AG wait; only feeds the readout)
        ns_A = nc.enter_named_scope("A_atoms", False)[0]
        with tc.tile_pool(name="pa", bufs=1) as pa, \
             tc.tile_pool(name="pa2", bufs=2) as pa2, \
             tc.tile_pool(name="pa_ps", bufs=4, space="PSUM") as paps:
            WqA_h = load_round(pa, wq_a[0:128, :], [128, F], "WqA_h", dt=f32)
            WqA_l = load_round(pa, wq_a[128:F, :], [5, F], "WqA_l", dt=f32)
            WkA_h = load_round(pa, wk_a[0:128, :], [128, F], "WkA_h", dt=f32)
            WkA_l = load_round(pa, wk_a[128:F, :], [5, F], "WkA_l", dt=f32)
            WvA_h = load_round(pa, wv_a[0:128, :], [128, F], "WvA_h", dt=f32)
            WvA_l = load_round(pa, wv_a[128:F, :], [5, F], "WvA_l", dt=f32)
            LnG = load_round(pa, ln_g[:], [1, F], "LnG", dt=f32)
            LnB = load_round(pa, ln_b[:], [1, F], "LnB", dt=f32)
            FaT_h = load_round(pa, faT[0:128, :], [128, AS], "FaT_h", dt=f32)
            FaT_l = load_round(pa, faT[128:F, :], [5, AS], "FaT_l", dt=f32)
            XN = pa.tile([64, MPC, F], f32, name="XN")
            nc.sync.dma_start(XN[:], fa[:].rearrange("(m a) f -> a m f", a=64))
            GB = pa.tile([64, F], f32, name="GB")
            BB = pa.tile([64, F], f32, name="BB")
            for bc_src, bc_dst in ((LnG, GB), (LnB, BB)):
                ps = paps.tile([64, F], f32, name="ps")
                nc.tensor.matmul(ps[:], OnesrF[:, 0:64], bc_src[:],
                                 start=True, stop=True)
                nc.vector.tensor_copy(bc_dst[:], ps[:])

            def mm133(dst, lhs_pair, rhs_pair, n):
                ps = paps.tile([dst.shape[0], n], f32, name="ps")
                nc.tensor.matmul(ps[:], lhs_pair[0], rhs_pair[0],
                                 start=True, stop=False)
                nc.tensor.matmul(ps[:], lhs_pair[1], rhs_pair[1],
                                 start=False, stop=True)
                nc.vector.tensor_copy(dst, ps[:])

            QTa_h = pa.tile([128, AS], f32, name="QTa_h")
            QTa_l = pa.tile([5, AS], f32, name="QTa_l")
            KTa_h = pa.tile([128, AS], f32, name="KTa_h")
            KTa_l = pa.tile([5, AS], f32, name="KTa_l")
            mm133(QTa_h[:], (WqA_h[:, 0:128], WqA_l[:, 0:128]),
                  (FaT_h[:], FaT_l[:]), AS)
            mm133(QTa_l[:], (WqA_h[:, 128:F], WqA_l[:, 128:F]),
                  (FaT_h[:], FaT_l[:]), AS)
            mm133(KTa_h[:], (WkA_h[:, 0:128], WkA_l[:, 0:128]),
                  (FaT_h[:], FaT_l[:]), AS)
            mm133(KTa_l[:], (WkA_h[:, 128:F], WkA_l[:, 128:F]),
                  (FaT_h[:], FaT_l[:]), AS)
            VN = pa.tile([64, MPC, F], f32, name="VN")
            for m in range(MPC):
                mm133(VN[:, m, :],
                      (FaT_h[:, bass.ts(m, 64)], FaT_l[:, bass.ts(m, 64)]),
                      (WvA_h[:], WvA_l[:]), F)
            E = pa.tile([64, MPC, 64], f32, name="E")
            for m in range(MPC):
                ps = paps.tile([64, 64], f32, name="ps")
                nc.tensor.matmul(ps[:], QTa_h[:, bass.ts(m, 64)],
                                 KTa_h[:, bass.ts(m, 64)], start=True, stop=False)
                nc.tensor.matmul(ps[:], QTa_l[:, bass.ts(m, 64)],
                                 KTa_l[:, bass.ts(m, 64)], start=False, stop=True)
                nc.scalar.activation(E[:, m, :], ps[:], AF.Exp, scale=SCALE_ATOM)
            SumsA = pa.tile([64, MPC, 1], f32, name="SumsA")
            RS = pa.tile([64, MPC, 1], f32, name="RS")
            nc.vector.tensor_reduce(SumsA[:], E[:], axis=AX.X, op=ALU.add)
            nc.vector.reciprocal(RS[:], SumsA[:])
            AttnN = pa.tile([64, MPC, F], f32, name="AttnN")
            for m in range(MPC):
                pst = paps.tile([64, 64], f32, name="ps")
                nc.tensor.transpose(pst[:], E[:, m, :], Ident[0:64, 0:64])
                ET = pa2.tile([64, 64], f32, name="ET")
                nc.vector.tensor_copy(ET[:], pst[:])
                ps = paps.tile([64, F], f32, name="ps")
                nc.tensor.matmul(ps[:], ET[:], VN[:, m, :], start=True, stop=True)
                nc.vector.tensor_scalar_mul(AttnN[:, m, :], ps[:], RS[:, m, :])
            SumX = pa.tile([64, MPC, F], f32, name="SumX")
            nc.vector.tensor_tensor(SumX[:], XN[:], AttnN[:], op=ALU.add)
            Mu = pa.tile([64, MPC, 1], f32, name="Mu")
            nc.vector.tensor_reduce(Mu[:], SumX[:], axis=AX.X, op=ALU.add)
            nc.vector.tensor_scalar_mul(Mu[:], Mu[:], 1.0 / F)
            XC = pa.tile([64, MPC, F], f32, name="XC")
            nc.vector.tensor_tensor(XC[:], SumX[:],
                                    Mu[:].to_broadcast([64, MPC, F]),
                                    op=ALU.subtract)
            SQ = pa.tile([64, MPC, F], f32, name="SQ")
            nc.vector.tensor_tensor(SQ[:], XC[:], XC[:], op=ALU.mult)
            Var = pa.tile([64, MPC, 1], f32, name="Var")
            nc.vector.tensor_reduce(Var[:], SQ[:], axis=AX.X, op=ALU.add)
            Std = pa.tile([64, MPC, 1], f32, name="Std")
            EpsT = pa.tile([64, 1], f32, name="EpsT")
            nc.vector.memset(EpsT[:], 1e-5)
            nc.scalar.activation(Std[:], Var[:], AF.Sqrt, scale=1.0 / F,
                                 bias=EpsT[:])
            RStd = pa.tile([64, MPC, 1], f32, name="RStd")
            nc.vector.reciprocal(RStd[:], Std[:])
            FeN = pa.tile([64, MPC, F], f32, name="FeN")
            nc.vector.tensor_tensor(XC[:], XC[:],
                                    RStd[:].to_broadcast([64, MPC, F]),
                                    op=ALU.mult)
            nc.vector.tensor_tensor(XC[:], XC[:],
                                    GB[:, None, :].to_broadcast([64, MPC, F]),
                                    op=ALU.mult)
            nc.vector.tensor_tensor(FeN[:], XC[:],
                                    BB[:, None, :].to_broadcast([64, MPC, F]),
                                    op=ALU.add)
            for m in range(MPC):
                ps1 = paps.tile([128, 64], f32, name="ps")
                nc.tensor.transpose(ps1[:], FeN[:, m, 0:128], Ident[0:64, 0:64])
                nc.vector.tensor_copy(FeT_h[:, bass.ts(m, 64)], ps1[:])
                ps2 = paps.tile([5, 64], f32, name="ps")
                nc.tensor.transpose(ps2[:], FeN[:, m, 128:F], Ident[0:64, 0:64])
                nc.vector.tensor_copy(FeT_l[:, bass.ts(m, 64)], ps2[:])
        nc.leave_named_scope("A_atoms", ns_A, False)

        # ============ Readout + per-molecule pooling =======================
        ns_D = nc.enter_named_scope("D_readout", False)[0]
        with tc.tile_pool(name="pd", bufs=1) as pd, \
             tc.tile_pool(name="pd2", bufs=2) as pd2, \
             tc.tile_pool(name="pd_ps", bufs=4, space="PSUM") as pdps:
            # a_message via batched row gathers + f32 tree-sum + transposes
            Gr = pd.tile([128, MNB, AS // 128, H], bf16, name="Gr")
            for k in range(MNB):
                for c in range(AS // 128):
                    nc.gpsimd.indirect_dma_start(
                        out=Gr[:, k, c, :], out_offset=None,
                        in_=agm_out[NITER][:],
                        in_offset=bass.IndirectOffsetOnAxis(
                            ap=RoIdx[:, k, c:c + 1], axis=0))
            A1 = pd.tile([128, AS // 128, H], f32, name="A1")
            A2 = pd.tile([128, AS // 128, H], f32, name="A2")
            gr = lambda k: Gr[:, k, :, :]
            nc.vector.tensor_tensor(A1[:], gr(0), gr(1), op=ALU.add)
            nc.vector.tensor_tensor(A2[:], gr(2), gr(3), op=ALU.add)
            nc.vector.tensor_tensor(A1[:], A1[:], A2[:], op=ALU.add)
            nc.vector.tensor_tensor(A2[:], gr(4), gr(5), op=ALU.add)
            nc.vector.tensor_tensor(A1[:], A1[:], A2[:], op=ALU.add)
            AmT2 = pd.tile([128, 2, AS], f32r, name="AmT2")
            for c in range(AS // 128):
                pst = pdps.tile([128, 512], f32, name="ps")
                for kt in range(2):
                    nc.tensor.transpose(pst[:, bass.ts(kt, 128)],
                                        A1[:, c, bass.ts(kt, 128)], Ident[:])
                nc.vector.tensor_copy(
                    AmT2[:, :, c * 128:(c + 1) * 128],
                    pst[:, 0:256].rearrange("p (t x) -> p t x", t=2))
            Hm = pd.tile([128, AS // 128, H], f32r, name="Hm")
            for c in range(AS // 128):
                ps = pdps.tile([128, H], f32, name="ps")
                nc.tensor.matmul(ps[:], FeT_h[:, bass.ts(c, 128)], Wof_h[:],
                                 start=True, stop=False)
                nc.tensor.matmul(ps[:], FeT_l[:, bass.ts(c, 128)], Wof_l[:],
                                 start=False, stop=False)
                for kt in range(2):
                    nc.tensor.matmul(ps[:], AmT2[:, kt, bass.ts(c, 128)],
                                     Wom[:, kt, :], start=False, stop=False)
                nc.tensor.matmul(ps[:], Onesr[:, 0:128], Wob[:],
                                 start=False, stop=True)
                nc.scalar.activation(Hm[:, c, :], ps[:], AF.Relu)
            HmT = pd.tile([128, 2, AS], f32r, name="HmT")
            for ht in range(2):
                pst = pdps.tile([128, AS], f32, name="ps")
                for c in range(AS // 128):
                    nc.tensor.transpose(pst[:, bass.ts(c, 128)],
                                        Hm[:, c, bass.ts(ht, 128)].bitcast(f32),
                                        Ident[:])
                nc.vector.tensor_copy(HmT[:, ht, :], pst[:])
            # hm in per-molecule base-0 layout via SBUF->SBUF DMA
            HmM = pd.tile([64, MPC, H], f32r, name="HmM")
            nc.sync.dma_start(HmM[:, 0:MPC:2, :], Hm[0:64, :, :])
            nc.sync.dma_start(HmM[:, 1:MPC:2, :], Hm[64:128, :, :])
            T2T = pd.tile([128, 2, AS], f32r, name="T2T")
            for ht in range(2):
                ps = pdps.tile([128, AS], f32, name="ps")
                for kt in range(2):
                    nc.tensor.matmul(ps[:], War[:, kt, bass.ts(ht, 128)],
                                     HmT[:, kt, :], start=(kt == 0), stop=(kt == 1))
                nc.vector.tensor_copy(T2T[:, ht, :], ps[:])
            SC2 = pd.tile([64, MPC, 64], f32, name="SC2")
            for m in range(MPC):
                ps = pdps.tile([64, 64], f32, name="ps")
                for kt in range(2):
                    nc.tensor.matmul(ps[:], T2T[:, kt, bass.ts(m, 64)],
                                     HmT[:, kt, bass.ts(m, 64)],
                                     start=(kt == 0), stop=(kt == 1))
                nc.vector.tensor_copy(SC2[:, m, :], ps[:])
            Mx2 = pd.tile([64, MPC, 1], f32, name="Mx2")
            nc.vector.tensor_reduce(Mx2[:], SC2[:], axis=AX.X, op=ALU.max)
            NMx2 = pd.tile([64, MPC, 1], f32, name="NMx2")
            nc.vector.tensor_scalar_mul(NMx2[:], Mx2[:], -1.0)
            E2 = pd.tile([64, MPC, 64], f32, name="E2")
            for m in range(MPC):
                nc.scalar.activation(E2[:, m, :], SC2[:, m, :], AF.Exp,
                                     bias=NMx2[:, m, :])
            Sum2 = pd.tile([64, MPC, 1], f32, name="Sum2")
            RS2 = pd.tile([64, MPC, 1], f32, name="RS2")
            nc.vector.tensor_reduce(Sum2[:], E2[:], axis=AX.X, op=ALU.add)
            nc.vector.reciprocal(RS2[:], Sum2[:])
            BB2 = pd.tile([64, H], f32, name="BB2")
            psbb = pdps.tile([64, H], f32, name="ps")
            nc.tensor.matmul(psbb[:], Onesr[:, 0:64], Wbb[:], start=True, stop=True)
            nc.vector.tensor_copy(BB2[:], psbb[:])
            OutS = pd.tile([1, MPC, H], f32, name="OutS")
            for m in range(MPC):
                pst = pdps.tile([64, 64], f32, name="ps")
                nc.tensor.transpose(pst[:], E2[:, m, :], Ident[0:64, 0:64])
                E2T = pd2.tile([64, 64], f32r, name="E2T")
                nc.vector.tensor_copy(E2T[:], pst[:])
                UT = pd2.tile([128, 2, 64], f32r, name="UT")
                for hs in range(2):
                    psu = pdps.tile([128, 64], f32, name="ps")
                    nc.tensor.matmul(psu[:], HmM[:, m, bass.ts(hs, 128)], E2T[:],
                                     start=True, stop=True)
                    nc.vector.tensor_copy(UT[:, hs, :], psu[:])
                psb = pdps.tile([64, H], f32, name="ps")
                for kt in range(2):
                    nc.tensor.matmul(psb[:], UT[:, kt, :], Wbw[:, kt, :],
                                     start=(kt == 0), stop=(kt == 1))
                AH = pd2.tile([64, H], f32, name="AH")
                nc.vector.tensor_scalar_mul(AH[:], psb[:], RS2[:, m, :])
                nc.vector.tensor_tensor(AH[:], AH[:], BB2[:], op=ALU.add)
                nc.vector.tensor_scalar_max(AH[:], AH[:], 0.0)
                XS = pd2.tile([64, H], f32r, name="XS")
                nc.vector.tensor_tensor(XS[:], AH[:], HmM[:, m, :], op=ALU.add)
                psm = pdps.tile([1, H], f32, name="ps")
                nc.tensor.matmul(psm[:], OnesC[0:64, :], XS[:],
                                 start=True, stop=True)
                nc.vector.tensor_scalar_mul(OutS[:, m, :], psm[:], 1.0 / APM)
            nc.sync.dma_start(out_mol[:].rearrange("(o m) h -> o m h", o=1), OutS[:])
        nc.leave_named_scope("D_readout", ns_D, False)

    nc.compile()
    return nc


def _host_prepare(inputs):
    f_atoms = np.asarray(inputs["f_atoms"], np.float32)
    f_bonds = np.asarray(inputs["f_bonds"], np.float32)
    a2b = np.asarray(inputs["a2b"]).astype(np.int64)
    b2a = np.asarray(inputs["b2a"]).astype(np.int64)
    b2revb = np.asarray(inputs["b2revb"]).astype(np.int64)

    fbp = np.zeros((B, H), np.float32)
    fbp[:, :FD] = f_bonds
    fbT_full = np.ascontiguousarray(fbp.T)
    faT_full = np.ascontiguousarray(f_atoms.T)

    W = {k: np.asarray(inputs[k], np.float32) for k in
         ("Wq_atom", "Wk_atom", "Wv_atom", "Wi", "Wh", "Wq", "Wk", "Wv", "Wa",
          "Walpha_w", "Wo_w", "Wb_w")}
    wi_p = np.zeros((H, H), np.float32)
    wi_p[:FD, :] = W["Wi"]
    wqk = np.ascontiguousarray(W["Wq"] @ W["Wk"].T)
    wva = np.ascontiguousarray(W["Wv"] @ W["Wa"])

    base = dict(
        wq_a=W["Wq_atom"], wk_a=W["Wk_atom"], wv_a=W["Wv_atom"],
        ln_g=np.asarray(inputs["ln_g"], np.float32).reshape(1, F),
        ln_b=np.asarray(inputs["ln_b"], np.float32).reshape(1, F),
        wi=wi_p, wqk=wqk, wh=W["Wh"], wva=wva, wa=W["Wa"],
        wal1=np.ascontiguousarray(W["Walpha_w"][:H]),
        wal2=np.ascontiguousarray(W["Walpha_w"][H:]),
        nwalb=-np.asarray(inputs["Walpha_b"], np.float32).reshape(1, 1),
        wo_f=np.ascontiguousarray(W["Wo_w"][:F]),
        wo_m=np.ascontiguousarray(W["Wo_w"][F:]),
        wo_b=np.asarray(inputs["Wo_b"], np.float32).reshape(1, H),
        wb_w=W["Wb_w"],
        wb_b=np.asarray(inputs["Wb_b"], np.float32).reshape(1, H),
        ident=np.eye(128, dtype=np.float32),
        onesr=np.ones((1, 128), np.float32),
        onesc=np.ones((128, 1), np.float32),
    )

    in_maps = []
    for c in range(NC):
        bonds = np.arange(c * BS, (c + 1) * BS)
        # [7, BS]: rows 0..5 = a2b[b2a] terms (added), row 6 = b2revb (subbed)
        terms = np.stack([a2b[b2a[bonds], j] for j in range(MNB)]
                         + [b2revb[bonds]])
        nb2 = np.ascontiguousarray(
            terms.reshape(7, BS // 128, 128).transpose(2, 0, 1)).astype(np.int32)
        atoms = np.arange(c * AS, (c + 1) * AS)
        ro = np.stack([a2b[atoms, j] for j in range(MNB)])  # [6, AS]
        ro2 = np.ascontiguousarray(
            ro.reshape(MNB, AS // 128, 128).transpose(2, 0, 1)).astype(np.int32)
        m = dict(base)
        m["fbT"] = np.ascontiguousarray(fbT_full[:, bonds])
        m["faT"] = np.ascontiguousarray(faT_full[:, atoms])
        m["fa"] = np.ascontiguousarray(f_atoms[atoms])
        m["nb_idx"] = nb2
        m["ro_idx"] = ro2
        in_maps.append(m)
    return in_maps


def kernel(**inputs):
    if "nc" not in _CACHE:
        _CACHE["nc"] = _build()
    nc = _CACHE["nc"]
    in_maps = _host_prepare(inputs)
    res = run_bass_kernel_spmd(nc, in_maps, core_ids=list(range(NC)))
    out = np.concatenate([res.results[c]["mol_out"] for c in range(NC)], 0)
    return np.ascontiguousarray(out.astype(np.float32))


# revision 43
# speedup vs baseline: 1.2834x; 1.2834x over previous
"""Trainium2 Bass kernel for nn_HGNNEncoder (DMPNN + global bond attention).

Sharding: data-parallel over bonds/atoms/molecules across 8 NeuronCores.

Key structure (v2):
 - Weight folding: scores = (M Wq)(M Wk)^T = M Wqk M^T with Wqk = Wq Wk^T,
   and att = (P V) Wa = (P M)(Wv Wa) = (P M) Wva.  So the attention needs
   only the raw message M as both K and V -> no K/V compute at all.
 - One AllGather per iteration boundary carrying [msgN | msgT] in bf16
   (1 MB per rank).  M^T (for QK lhsT) and M-natural+ones (for PV rhs)
   are DMA'd back from the AG output.
 - DMPNN neighbor-sum via one dma_gather(transpose=True) per iteration
   (7*1024 rows, bf16) + DVE tree-sum -> directly transposed T^T.
 - bf16 everywhere on the matmul paths, fp32 PSUM accumulation, exp on
   ScalarE with the 1/16 scale folded in, sigmoid computed as
   1/(1+exp(-x)) to avoid ACT table swaps.
"""

import numpy as np

import concourse.bass as bass
import concourse.bacc as bacc
import concourse.mybir as mybir
import concourse.tile as tile
from concourse.bass_utils import run_bass_kernel_spmd

NC = 8
B, NA, MNB = 8192, 4096, 6
H = 256
F = 133
FD = 147
BS = B // NC          # 1024 bonds per core
AS = NA // NC         # 512 atoms per core
APM = 64              # atoms per molecule
MPC = AS // APM       # 8 molecules per core
NITER = 3
NBI = 7 * BS          # 7168 dmpnn gather idxs
ROI = MNB * AS        # 3072 readout gather idxs
PAYR = 2 * BS         # 2048 rows of 256 per rank in the AG payload

f32 = mybir.dt.float32
f32r = mybir.dt.float32r
bf16 = mybir.dt.bfloat16
i16 = mybir.dt.int16
AF = mybir.ActivationFunctionType
ALU = mybir.AluOpType
AX = mybir.AxisListType

SCALE_BOND = float(1.0 / np.sqrt(np.float32(H)))
SCALE_ATOM = float(1.0 / np.sqrt(np.float32(F)))

_CACHE = {}


def _build():
    nc = bacc.Bacc("TRN2", target_bir_lowering=False, debug=False, num_devices=NC)

    def inp(name, shape, dt=f32):
        return nc.dram_tensor(name, list(shape), dt, kind="ExternalInput")

    fbT = inp("fbT", [H, BS])
    faT = inp("faT", [F, AS])
    fa = inp("fa", [AS, F])
    wq_a = inp("wq_a", [F, F]); wk_a = inp("wk_a", [F, F]); wv_a = inp("wv_a", [F, F])
    ln_g = inp("ln_g", [1, F]); ln_b = inp("ln_b", [1, F])
    wi = inp("wi", [H, H])
    wqk = inp("wqk", [H, H]); wh = inp("wh", [H, H]); wva = inp("wva", [H, H])
    wa = inp("wa", [H, H])
    wal1 = inp("wal1", [H, 1]); wal2 = inp("wal2", [H, 1]); nwalb = inp("nwalb", [1, 1])
    wo_f = inp("wo_f", [F, H]); wo_m = inp("wo_m", [H, H]); wo_b = inp("wo_b", [1, H])
    wb_w = inp("wb_w", [H, H]); wb_b = inp("wb_b", [1, H])
    ident_in = inp("ident", [128, 128])
    onesr_in = inp("onesr", [1, 128])
    onesc_in = inp("onesc", [128, 1])
    nb_idx = inp("nb_idx", [128, 7, BS // 128], mybir.dt.int32)
    ro_idx = inp("ro_idx", [128, MNB, AS // 128], mybir.dt.int32)
    out_mol = nc.dram_tensor("mol_out", [MPC, H], f32, kind="ExternalOutput")

    with tile.TileContext(nc) as tc, \
         tc.tile_pool(name="persist", bufs=1) as per, \
         tc.tile_pool(name="dram", bufs=1, space="DRAM") as dram, \
         tc.tile_pool(name="psA", bufs=2, space="PSUM") as ppsA:

        def load_round(pool, src_ap, shape, name, dt=f32r, raw_pool=None):
            raw = (raw_pool or pool).tile(list(shape), f32, name=name + "_raw")
            nc.sync.dma_start(raw[:], src_ap)
            if dt == f32:
                return raw
            t = pool.tile(list(shape), dt, name=name)
            nc.vector.tensor_copy(t[:], raw[:])
            return t

        # ---------------- persistent weights ----------------
        with tc.tile_pool(name="raws", bufs=1) as raws:
            def loadw(src, name, cols=H, dt=bf16):
                return load_round(per, src[:].rearrange("(t p) h -> p t h", p=128),
                                  [128, 2, cols], name, dt=dt, raw_pool=raws)
            Wi = loadw(wi, "Wi", dt=f32r); Wqk = loadw(wqk, "Wqk", dt=f32r)
            Wh = loadw(wh, "Wh", dt=f32r)
            Wva = loadw(wva, "Wva"); Wom = loadw(wo_m, "Wom", dt=f32r)
            War = loadw(wa, "War", dt=f32r); Wbw = loadw(wb_w, "Wbw", dt=f32r)
            Wal1 = loadw(wal1, "Wal1", cols=1, dt=f32r)
            Wal2 = loadw(wal2, "Wal2", cols=1, dt=f32r)
            Wof_h = load_round(per, wo_f[0:128, :], [128, H], "Wof_h", raw_pool=raws)
            Wof_l = load_round(per, wo_f[128:F, :], [5, H], "Wof_l", raw_pool=raws)
            Wob = load_round(per, wo_b[:], [1, H], "Wob", raw_pool=raws)
            Wbb = load_round(per, wb_b[:], [1, H], "Wbb", raw_pool=raws)
            Ident = load_round(per, ident_in[:], [128, 128], "Ident", dt=f32)
            IdentB = load_round(per, ident_in[:], [128, 128], "IdentB", dt=bf16,
                                raw_pool=raws)
            Onesr = load_round(per, onesr_in[:], [1, 128], "Onesr", raw_pool=raws)
            OnesC = load_round(per, onesc_in[:], [128, 1], "OnesC", raw_pool=raws)
            NWalb = load_round(per, nwalb[:], [1, 1], "NWalb", dt=f32)
            OnesrF = load_round(per, onesr_in[:], [1, 128], "OnesrF", dt=f32)
            NbIdx = per.tile([128, 7, BS // 128], mybir.dt.int32, name="NbIdx")
            nc.sync.dma_start(NbIdx[:], nb_idx[:])
            RoIdx = per.tile([128, MNB, AS // 128], mybir.dt.int32, name="RoIdx")
            nc.sync.dma_start(RoIdx[:], ro_idx[:])

        # persistent activations
        InpT = per.tile([128, 2, BS], f32, name="InpT")
        MsgTx = per.tile([128, 2, BS], bf16, name="MsgTx")
        MsgTxF = per.tile([128, 2, BS], f32r, name="MsgTxF")
        MsgN = per.tile([128, BS // 128, H], bf16, name="MsgN")
        FeT_h = per.tile([128, AS], f32r, name="FeT_h")
        FeT_l = per.tile([5, AS], f32r, name="FeT_l")

        agx_in = [dram.tile([PAYR, H], bf16, name=f"agx_in{t}") for t in range(NITER)]
        agx_out = [dram.tile([NC * PAYR, H], bf16, name=f"agx_out{t}",
                             addr_space="Shared") for t in range(NITER)]
        agn_in = dram.tile([BS, H], bf16, name="agn_in")
        agn_out = dram.tile([B, H], bf16, name="agn_out", addr_space="Shared")
        RG = [list(range(NC))]

        def boundary(it):
            # natural-layout local message via PE transposes
            for c in range(BS // 128):
                psn = ppsA.tile([128, 512], f32, name="psA")[:].bitcast(bf16)
                for ht in range(2):
                    nc.tensor.transpose(psn[:, bass.ts(ht, 128)],
                                        MsgTx[:, ht, bass.ts(c, 128)], IdentB[:])
                nc.vector.tensor_copy(MsgN[:, c, :], psn[:, 0:H])
            if it < NITER:
                nc.sync.dma_start(
                    agx_in[it][0:BS, :].rearrange("(lb p) e -> p lb e", p=128),
                    MsgN[:])
                nc.sync.dma_start(
                    agx_in[it][BS:PAYR, :].rearrange("(t p jj) e -> p t (jj e)",
                                                     t=2, p=128),
                    MsgTx[:])
                nc.gpsimd.collective_compute(
                    "AllGather", ALU.bypass, RG,
                    ins=[agx_in[it].opt()], outs=[agx_out[it].opt()])
            else:
                nc.sync.dma_start(
                    agn_in[:].rearrange("(lb p) e -> p lb e", p=128), MsgN[:])
                nc.gpsimd.collective_compute(
                    "AllGather", ALU.bypass, RG,
                    ins=[agn_in.opt()], outs=[agn_out.opt()])

        def qt_prime():
            # Q'^T = Wqk^T @ M^T  (local shard, f32r for precision)
            for ht in range(2):
                for ch in range(2):
                    ps = ppsA.tile([128, 512], f32, name="psA")
                    for kt in range(2):
                        nc.tensor.matmul(
                            ps[:], Wqk[:, kt, bass.ts(ht, 128)],
                            MsgTxF[:, kt, bass.ts(ch, 512)],
                            start=(kt == 0), stop=(kt == 1))
                    nc.vector.tensor_copy(QT[:, ht, bass.ts(ch, 512)], ps[:])

        # ============ Phase B: message_0 = relu(f_bonds @ Wi) ==============
        ns_B = nc.enter_named_scope("B_init", False)[0]
        with tc.tile_pool(name="pb", bufs=1) as pb:
            FbT = load_round(pb, fbT[:].rearrange("(t p) i -> p t i", p=128),
                             [128, 2, BS], "FbT")
            for ht in range(2):
                for ch in range(2):
                    ps = ppsA.tile([128, 512], f32, name="psA")
                    for kt in range(2):
                        nc.tensor.matmul(ps[:], Wi[:, kt, bass.ts(ht, 128)],
                                         FbT[:, kt, bass.ts(ch, 512)],
                                         start=(kt == 0), stop=(kt == 1))
                    nc.vector.tensor_copy(InpT[:, ht, bass.ts(ch, 512)], ps[:])
                    nc.scalar.activation(MsgTx[:, ht, bass.ts(ch, 512)], ps[:],
                                         AF.Relu)
                    nc.scalar.activation(MsgTxF[:, ht, bass.ts(ch, 512)], ps[:],
                                         AF.Relu)
            boundary(0)
        nc.leave_named_scope("B_init", ns_B, False)

        # ============ Phase C: 3 message-passing iterations ================
        with tc.tile_pool(name="psS", bufs=2, space="PSUM") as ppsS, \
             tc.tile_pool(name="psP", bufs=1, space="PSUM") as ppsP, \
             tc.tile_pool(name="pmt", bufs=2) as pmt, \
             tc.tile_pool(name="pcw", bufs=1) as pcw:
            QT = pcw.tile([128, 2, BS], bf16, name="QT")
            MT = pcw.tile([128, 2, NC, BS], bf16, name="MT")
            Mb = pcw.tile([128, B // 128, H + 1], bf16, name="Mb")
            Gn = pcw.tile([128, 7, BS // 128, H], bf16, name="Gn")
            T1 = pcw.tile([128, BS // 128, H], f32, name="T1")
            TT = pcw.tile([128, 2, BS], f32r, name="TT")
            DmT = pcw.tile([128, 2, BS], f32r, name="DmT")
            AmT = pcw.tile([128, 2, BS], f32r, name="AmT")
            AtT = pcw.tile([128, 2, BS], bf16, name="AtT")
            AttA = pcw.tile([128, BS // 128, H], bf16, name="AttA")
            AlB = pcw.tile([128, BS], f32, name="AlB")
            Alp = pcw.tile([1, BS], f32r, name="Alp")
            Ea = pcw.tile([1, BS], f32, name="Ea")
            D1 = pcw.tile([128, BS], f32, name="D1")
            nc.vector.memset(Mb[:], 1.0)   # ones column persists across iters
            qt_prime()
            for it in range(NITER):
                ns_it = nc.enter_named_scope(f"C{it}", False)[0]
                src = agx_out[it]
                # ---- DMA-in M^T and M-natural(+ones) from the AG output
                for c in range(NC):
                    nc.sync.dma_start(
                        MT[:, :, c, :],
                        src[c * PAYR + BS:(c + 1) * PAYR, :]
                        .rearrange("(t p jj) e -> p t (jj e)", t=2, p=128))
                for c in range(NC):
                    nc.sync.dma_start(
                        Mb[:, c * 8:(c + 1) * 8, 0:H],
                        src[c * PAYR:c * PAYR + BS, :]
                        .rearrange("(lb p) e -> p lb e", p=128))
                # ---- dmpnn gather (batched rows); sums/transposes after attn
                ns_g = nc.enter_named_scope(f"C{it}_gather", False)[0]
                for k in range(7):
                    for c in range(BS // 128):
                        nc.gpsimd.indirect_dma_start(
                            out=Gn[:, k, c, :], out_offset=None, in_=src[:],
                            in_offset=bass.IndirectOffsetOnAxis(
                                ap=NbIdx[:, k, c:c + 1], axis=0))
                nc.leave_named_scope(f"C{it}_gather", ns_g, False)

                # ---- attention (rows = shard), flash-style over j blocks
                ns_at = nc.enter_named_scope(f"C{it}_attn", False)[0]
                for ic in range(2):
                    pvp = ppsP.tile([128, 4, 512], f32, name="psP")
                    prev = None
                    for jj in range(B // 128 + 1):
                        if jj < B // 128:
                            c, jl = divmod(jj, 8)
                            sp = ppsS.tile([128, 512], f32, name="psS")
                            for kt in range(2):
                                nc.tensor.matmul(
                                    sp[:], MT[:, kt, c, bass.ts(jl, 128)],
                                    QT[:, kt, bass.ts(ic, 512)],
                                    start=(kt == 0), stop=(kt == 1))
                            pt = pmt.tile([128, 512], bf16, name="PT")
                            nc.scalar.activation(pt[:], sp[:], AF.Exp,
                                                 scale=SCALE_BOND)
                        if prev is not None:
                            pj, ppt = prev
                            for isub in range(4):
                                nc.tensor.matmul(
                                    pvp[:, isub, 0:H + 1],
                                    ppt[:, bass.ts(isub, 128)],
                                    Mb[:, pj, 0:H + 1],
                                    start=(pj == 0), stop=(pj == B // 128 - 1))
                        if jj < B // 128:
                            prev = (jj, pt)
                    for isub in range(4):
                        rec = pmt.tile([128, 1], f32, name="rec")
                        nc.vector.reciprocal(rec[:], pvp[:, isub, H:H + 1])
                        nc.vector.tensor_scalar_mul(
                            AttA[:, ic * 4 + isub, :], pvp[:, isub, 0:H], rec[:])
                nc.leave_named_scope(f"C{it}_attn", ns_at, False)
                ns_po = nc.enter_named_scope(f"C{it}_post", False)[0]

                # ---- tree-sum + transposes (PE queue: after the attention)
                g = lambda k: Gn[:, k, :, :]
                # tree-sum scratch aliases AmT (written later this iteration)
                T2v = AmT[:].rearrange("p t (c e) -> p (t c) e", e=256)
                nc.vector.tensor_tensor(T1[:], g(0), g(1), op=ALU.add)
                nc.vector.tensor_tensor(T2v, g(2), g(3), op=ALU.add)
                nc.vector.tensor_tensor(T1[:], T1[:], T2v, op=ALU.add)
                nc.vector.tensor_tensor(T2v, g(4), g(5), op=ALU.add)
                nc.vector.tensor_tensor(T1[:], T1[:], T2v, op=ALU.add)
                nc.vector.tensor_scalar_mul(T2v, g(6), -1.0)
                nc.vector.tensor_tensor(T1[:], T1[:], T2v, op=ALU.add)
                for c in range(BS // 128):
                    pst = ppsA.tile([128, 512], f32, name="psA")
                    for kt in range(2):
                        nc.tensor.transpose(pst[:, bass.ts(kt, 128)],
                                            T1[:, c, bass.ts(kt, 128)], Ident[:])
                    nc.vector.tensor_copy(
                        TT[:, :, c * 128:(c + 1) * 128],
                        pst[:, 0:256].rearrange("p (t x) -> p t x", t=2))

                # ---- dmpnn matmul (PE, after attention in queue order)
                for ht in range(2):
                    for ch in range(2):
                        ps = ppsA.tile([128, 512], f32, name="psA")
                        for kt in range(2):
                            nc.tensor.matmul(ps[:], Wh[:, kt, bass.ts(ht, 128)],
                                             TT[:, kt, bass.ts(ch, 512)],
                                             start=(kt == 0), stop=(kt == 1))
                        nc.vector.tensor_copy(DmT[:, ht, bass.ts(ch, 512)], ps[:])

                # ---- att^T then att_msg^T = Wva^T @ att^T
                for ht in range(2):
                    for half in range(2):
                        pst = ppsA.tile([128, 512], f32,
                                        name="psA")[:].bitcast(bf16)
                        for cc in range(4):
                            nc.tensor.transpose(
                                pst[:, bass.ts(cc, 128)],
                                AttA[:, half * 4 + cc, bass.ts(ht, 128)],
                                IdentB[:])
                        nc.vector.tensor_copy(AtT[:, ht, bass.ts(half, 512)],
                                              pst[:, 0:512])
                for ht in range(2):
                    for ch in range(2):
                        ps = ppsA.tile([128, 512], f32, name="psA")
                        for kt in range(2):
                            nc.tensor.matmul(ps[:], Wva[:, kt, bass.ts(ht, 128)],
                                             AtT[:, kt, bass.ts(ch, 512)],
                                             start=(kt == 0), stop=(kt == 1))
                        nc.vector.tensor_copy(AmT[:, ht, bass.ts(ch, 512)], ps[:])

                # ---- alpha = sigmoid(w1.dm + w2.am + b) via exp on ScalarE
                for ch in range(2):
                    ps = ppsA.tile([1, 512], f32, name="psA")
                    nc.tensor.matmul(ps[:], Wal1[:, 0, :],
                                     DmT[:, 0, bass.ts(ch, 512)],
                                     start=True, stop=False)
                    nc.tensor.matmul(ps[:], Wal1[:, 1, :],
                                     DmT[:, 1, bass.ts(ch, 512)],
                                     start=False, stop=False)
                    nc.tensor.matmul(ps[:], Wal2[:, 0, :],
                                     AmT[:, 0, bass.ts(ch, 512)],
                                     start=False, stop=False)
                    nc.tensor.matmul(ps[:], Wal2[:, 1, :],
                                     AmT[:, 1, bass.ts(ch, 512)],
                                     start=False, stop=True)
                    nc.scalar.activation(Ea[:, bass.ts(ch, 512)], ps[:],
                                         AF.Exp, scale=-1.0, bias=NWalb[:])
                nc.vector.tensor_scalar_add(Ea[:], Ea[:], 1.0)
                with nc.allow_low_precision(reason="f32r alpha"):
                    nc.vector.reciprocal(Alp[:], Ea[:])
                for ch in range(2):
                    ps = ppsA.tile([128, 512], f32, name="psA")
                    nc.tensor.matmul(ps[:], Onesr[:],
                                     Alp[:, bass.ts(ch, 512)],
                                     start=True, stop=True)
                    nc.vector.tensor_copy(AlB[:, bass.ts(ch, 512)], ps[:])

                # ---- combine (f32)
                for ht in range(2):
                    nc.vector.tensor_tensor(D1[:], DmT[:, ht, :],
                                            AmT[:, ht, :], op=ALU.subtract)
                    nc.vector.tensor_tensor(D1[:], D1[:], AlB[:], op=ALU.mult)
                    nc.vector.tensor_tensor(D1[:], D1[:], AmT[:, ht, :],
                                            op=ALU.add)
                    nc.vector.tensor_tensor(D1[:], D1[:], InpT[:, ht, :],
                                            op=ALU.add)
                    nc.scalar.activation(MsgTx[:, ht, :], D1[:], AF.Relu)
                    nc.scalar.activation(MsgTxF[:, ht, :], D1[:], AF.Relu)
                boundary(it + 1)
                if it + 1 < NITER:
                    qt_prime()
                nc.leave_named_scope(f"C{it}_post", ns_po, False)
                nc.leave_named_scope(f"C{it}", ns_it, False)

        # ============ Phase A: per-molecule atom self-attention ============
        # (emitted late: fills the final-AG wait; only feeds the readout)
        ns_A = nc.enter_named_scope("A_atoms", False)[0]
        with tc.tile_pool(name="pa", bufs=1) as pa, \
             tc.tile_pool(name="pa2", bufs=2) as pa2, \
             tc.tile_pool(name="pa_ps", bufs=4, space="PSUM") as paps:
            WqA_h = load_round(pa, wq_a[0:128, :], [128, F], "WqA_h", dt=f32)
            WqA_l = load_round(pa, wq_a[128:F, :], [5, F], "WqA_l", dt=f32)
            WkA_h = load_round(pa, wk_a[0:128, :], [128, F], "WkA_h", dt=f32)
            WkA_l = load_round(pa, wk_a[128:F, :], [5, F], "WkA_l", dt=f32)
            WvA_h = load_round(pa, wv_a[0:128, :], [128, F], "WvA_h", dt=f32)
            WvA_l = load_round(pa, wv_a[128:F, :], [5, F], "WvA_l", dt=f32)
            LnG = load_round(pa, ln_g[:], [1, F], "LnG", dt=f32)
            LnB = load_round(pa, ln_b[:], [1, F], "LnB", dt=f32)
            FaT_h = load_round(pa, faT[0:128, :], [128, AS], "FaT_h", dt=f32)
            FaT_l = load_round(pa, faT[128:F, :], [5, AS], "FaT_l", dt=f32)
            XN = pa.tile([64, MPC, F], f32, name="XN")
            nc.sync.dma_start(XN[:], fa[:].rearrange("(m a) f -> a m f", a=64))
            GB = pa.tile([64, F], f32, name="GB")
            BB = pa.tile([64, F], f32, name="BB")
            for bc_src, bc_dst in ((LnG, GB), (LnB, BB)):
                ps = paps.tile([64, F], f32, name="ps")
                nc.tensor.matmul(ps[:], OnesrF[:, 0:64], bc_src[:],
                                 start=True, stop=True)
                nc.vector.tensor_copy(bc_dst[:], ps[:])

            def mm133(dst, lhs_pair, rhs_pair, n):
                ps = paps.tile([dst.shape[0], n], f32, name="ps")
                nc.tensor.matmul(ps[:], lhs_pair[0], rhs_pair[0],
                                 start=True, stop=False)
                nc.tensor.matmul(ps[:], lhs_pair[1], rhs_pair[1],
                                 start=False, stop=True)
                nc.vector.tensor_copy(dst, ps[:])

            QTa_h = pa.tile([128, AS], f32, name="QTa_h")
            QTa_l = pa.tile([5, AS], f32, name="QTa_l")
            KTa_h = pa.tile([128, AS], f32, name="KTa_h")
            KTa_l = pa.tile([5, AS], f32, name="KTa_l")
            mm133(QTa_h[:], (WqA_h[:, 0:128], WqA_l[:, 0:128]),
                  (FaT_h[:], FaT_l[:]), AS)
            mm133(QTa_l[:], (WqA_h[:, 128:F], WqA_l[:, 128:F]),
                  (FaT_h[:], FaT_l[:]), AS)
            mm133(KTa_h[:], (WkA_h[:, 0:128], WkA_l[:, 0:128]),
                  (FaT_h[:], FaT_l[:]), AS)
            mm133(KTa_l[:], (WkA_h[:, 128:F], WkA_l[:, 128:F]),
                  (FaT_h[:], FaT_l[:]), AS)
            VN = pa.tile([64, MPC, F], f32, name="VN")
            for m in range(MPC):
                mm133(VN[:, m, :],
                      (FaT_h[:, bass.ts(m, 64)], FaT_l[:, bass.ts(m, 64)]),
                      (WvA_h[:], WvA_l[:]), F)
            E = pa.tile([64, MPC, 64], f32, name="E")
            for m in range(MPC):
                ps = paps.tile([64, 64], f32, name="ps")
                nc.tensor.matmul(ps[:], QTa_h[:, bass.ts(m, 64)],
                                 KTa_h[:, bass.ts(m, 64)], start=True, stop=False)
                nc.tensor.matmul(ps[:], QTa_l[:, bass.ts(m, 64)],
                                 KTa_l[:, bass.ts(m, 64)], start=False, stop=True)
                nc.scalar.activation(E[:, m, :], ps[:], AF.Exp, scale=SCALE_ATOM)
            SumsA = pa.tile([64, MPC, 1], f32, name="SumsA")
            RS = pa.tile([64, MPC, 1], f32, name="RS")
            nc.vector.tensor_reduce(SumsA[:], E[:], axis=AX.X, op=ALU.add)
            nc.vector.reciprocal(RS[:], SumsA[:])
            AttnN = pa.tile([64, MPC, F], f32, name="AttnN")
            for m in range(MPC):
                pst = paps.tile([64, 64], f32, name="ps")
                nc.tensor.transpose(pst[:], E[:, m, :], Ident[0:64, 0:64])
                ET = pa2.tile([64, 64], f32, name="ET")
                nc.vector.tensor_copy(ET[:], pst[:])
                ps = paps.tile([64, F], f32, name="ps")
                nc.tensor.matmul(ps[:], ET[:], VN[:, m, :], start=True, stop=True)
                nc.vector.tensor_scalar_mul(AttnN[:, m, :], ps[:], RS[:, m, :])
            SumX = pa.tile([64, MPC, F], f32, name="SumX")
            nc.vector.tensor_tensor(SumX[:], XN[:], AttnN[:], op=ALU.add)
            Mu = pa.tile([64, MPC, 1], f32, name="Mu")
            nc.vector.tensor_reduce(Mu[:], SumX[:], axis=AX.X, op=ALU.add)
            nc.vector.tensor_scalar_mul(Mu[:], Mu[:], 1.0 / F)
            XC = pa.tile([64, MPC, F], f32, name="XC")
            nc.vector.tensor_tensor(XC[:], SumX[:],
                                    Mu[:].to_broadcast([64, MPC, F]),
                                    op=ALU.subtract)
            SQ = pa.tile([64, MPC, F], f32, name="SQ")
            nc.vector.tensor_tensor(SQ[:], XC[:], XC[:], op=ALU.mult)
            Var = pa.tile([64, MPC, 1], f32, name="Var")
            nc.vector.tensor_reduce(Var[:], SQ[:], axis=AX.X, op=ALU.add)
            Std = pa.tile([64, MPC, 1], f32, name="Std")
            EpsT = pa.tile([64, 1], f32, name="EpsT")
            nc.vector.memset(EpsT[:], 1e-5)
            nc.scalar.activation(Std[:], Var[:], AF.Sqrt, scale=1.0 / F,
                                 bias=EpsT[:])
            RStd = pa.tile([64, MPC, 1], f32, name="RStd")
            nc.vector.reciprocal(RStd[:], Std[:])
            FeN = pa.tile([64, MPC, F], f32, name="FeN")
            nc.vector.tensor_tensor(XC[:], XC[:],
                                    RStd[:].to_broadcast([64, MPC, F]),
                                    op=ALU.mult)
            nc.vector.tensor_tensor(XC[:], XC[:],
                                    GB[:, None, :].to_broadcast([64, MPC, F]),
                                    op=ALU.mult)
            nc.vector.tensor_tensor(FeN[:], XC[:],
                                    BB[:, None, :].to_broadcast([64, MPC, F]),
                                    op=ALU.add)
            for m in range(MPC):
                ps1 = paps.tile([128, 64], f32, name="ps")
                nc.tensor.transpose(ps1[:], FeN[:, m, 0:128], Ident[0:64, 0:64])
                nc.vector.tensor_copy(FeT_h[:, bass.ts(m, 64)], ps1[:])
                ps2 = paps.tile([5, 64], f32, name="ps")
                nc.tensor.transpose(ps2[:], FeN[:, m, 128:F], Ident[0:64, 0:64])
                nc.vector.tensor_copy(FeT_l[:, bass.ts(m, 64)], ps2[:])
        nc.leave_named_scope("A_atoms", ns_A, False)

        # ============ Readout + per-molecule pooling =======================
        ns_D = nc.enter_named_scope("D_readout", False)[0]
        with tc.tile_pool(name="pd", bufs=1) as pd, \
             tc.tile_pool(name="pd2", bufs=2) as pd2, \
             tc.tile_pool(name="pd_ps", bufs=4, space="PSUM") as pdps:
            # a_message via batched row gathers + f32 tree-sum + transposes
            Gr = pd.tile([128, MNB, AS // 128, H], bf16, name="Gr")
            for k in range(MNB):
                for c in range(AS // 128):
                    nc.gpsimd.indirect_dma_start(
                        out=Gr[:, k, c, :], out_offset=None,
                        in_=agn_out[:],
                        in_offset=bass.IndirectOffsetOnAxis(
                            ap=RoIdx[:, k, c:c + 1], axis=0))
            A1 = pd.tile([128, AS // 128, H], f32, name="A1")
            A2 = pd.tile([128, AS // 128, H], f32, name="A2")
            gr = lambda k: Gr[:, k, :, :]
            nc.vector.tensor_tensor(A1[:], gr(0), gr(1), op=ALU.add)
            nc.vector.tensor_tensor(A2[:], gr(2), gr(3), op=ALU.add)
            nc.vector.tensor_tensor(A1[:], A1[:], A2[:], op=ALU.add)
            nc.vector.tensor_tensor(A2[:], gr(4), gr(5), op=ALU.add)
            nc.vector.tensor_tensor(A1[:], A1[:], A2[:], op=ALU.add)
            AmT2 = pd.tile([128, 2, AS], f32r, name="AmT2")
            for c in range(AS // 128):
                pst = pdps.tile([128, 512], f32, name="ps")
                for kt in range(2):
                    nc.tensor.transpose(pst[:, bass.ts(kt, 128)],
                                        A1[:, c, bass.ts(kt, 128)], Ident[:])
                nc.vector.tensor_copy(
                    AmT2[:, :, c * 128:(c + 1) * 128],
                    pst[:, 0:256].rearrange("p (t x) -> p t x", t=2))
            Hm = pd.tile([128, AS // 128, H], f32r, name="Hm")
            for c in range(AS // 128):
                ps = pdps.tile([128, H], f32, name="ps")
                nc.tensor.matmul(ps[:], FeT_h[:, bass.ts(c, 128)], Wof_h[:],
                                 start=True, stop=False)
                nc.tensor.matmul(ps[:], FeT_l[:, bass.ts(c, 128)], Wof_l[:],
                                 start=False, stop=False)
                for kt in range(2):
                    nc.tensor.matmul(ps[:], AmT2[:, kt, bass.ts(c, 128)],
                                     Wom[:, kt, :], start=False, stop=False)
                nc.tensor.matmul(ps[:], Onesr[:, 0:128], Wob[:],
                                 start=False, stop=True)
                nc.scalar.activation(Hm[:, c, :], ps[:], AF.Relu)
            HmT = pd.tile([128, 2, AS], f32r, name="HmT")
            for ht in range(2):
                pst = pdps.tile([128, AS], f32, name="ps")
                for c in range(AS // 128):
                    nc.tensor.transpose(pst[:, bass.ts(c, 128)],
                                        Hm[:, c, bass.ts(ht, 128)].bitcast(f32),
                                        Ident[:])
                nc.vector.tensor_copy(HmT[:, ht, :], pst[:])
            # hm in per-molecule base-0 layout via SBUF->SBUF DMA
            HmM = pd.tile([64, MPC, H], f32r, name="HmM")
            nc.sync.dma_start(HmM[:, 0:MPC:2, :], Hm[0:64, :, :])
            nc.sync.dma_start(HmM[:, 1:MPC:2, :], Hm[64:128, :, :])
            T2T = pd.tile([128, 2, AS], f32r, name="T2T")
            for ht in range(2):
                ps = pdps.tile([128, AS], f32, name="ps")
                for kt in range(2):
                    nc.tensor.matmul(ps[:], War[:, kt, bass.ts(ht, 128)],
                                     HmT[:, kt, :], start=(kt == 0), stop=(kt == 1))
                nc.vector.tensor_copy(T2T[:, ht, :], ps[:])
            SC2 = pd.tile([64, MPC, 64], f32, name="SC2")
            for m in range(MPC):
                ps = pdps.tile([64, 64], f32, name="ps")
                for kt in range(2):
                    nc.tensor.matmul(ps[:], T2T[:, kt, bass.ts(m, 64)],
                                     HmT[:, kt, bass.ts(m, 64)],
                                     start=(kt == 0), stop=(kt == 1))
                nc.vector.tensor_copy(SC2[:, m, :], ps[:])
            Mx2 = pd.tile([64, MPC, 1], f32, name="Mx2")
            nc.vector.tensor_reduce(Mx2[:], SC2[:], axis=AX.X, op=ALU.max)
            NMx2 = pd.tile([64, MPC, 1], f32, name="NMx2")
            nc.vector.tensor_scalar_mul(NMx2[:], Mx2[:], -1.0)
            E2 = pd.tile([64, MPC, 64], f32, name="E2")
            for m in range(MPC):
                nc.scalar.activation(E2[:, m, :], SC2[:, m, :], AF.Exp,
                                     bias=NMx2[:, m, :])
            Sum2 = pd.tile([64, MPC, 1], f32, name="Sum2")
            RS2 = pd.tile([64, MPC, 1], f32, name="RS2")
            nc.vector.tensor_reduce(Sum2[:], E2[:], axis=AX.X, op=ALU.add)
            nc.vector.reciprocal(RS2[:], Sum2[:])
            BB2 = pd.tile([64, H], f32, name="BB2")
            psbb = pdps.tile([64, H], f32, name="ps")
            nc.tensor.matmul(psbb[:], Onesr[:, 0:64], Wbb[:], start=True, stop=True)
            nc.vector.tensor_copy(BB2[:], psbb[:])
            OutS = pd.tile([1, MPC, H], f32, name="OutS")
            for m in range(MPC):
                pst = pdps.tile([64, 64], f32, name="ps")
                nc.tensor.transpose(pst[:], E2[:, m, :], Ident[0:64, 0:64])
                E2T = pd2.tile([64, 64], f32r, name="E2T")
                nc.vector.tensor_copy(E2T[:], pst[:])
                UT = pd2.tile([128, 2, 64], f32r, name="UT")
                for hs in range(2):
                    psu = pdps.tile([128, 64], f32, name="ps")
                    nc.tensor.matmul(psu[:], HmM[:, m, bass.ts(hs, 128)], E2T[:],
                                     start=True, stop=True)
                    nc.vector.tensor_copy(UT[:, hs, :], psu[:])
                psb = pdps.tile([64, H], f32, name="ps")
                for kt in range(2):
                    nc.tensor.matmul(psb[:], UT[:, kt, :], Wbw[:, kt, :],
                                     start=(kt == 0), stop=(kt == 1))
                AH = pd2.tile([64, H], f32, name="AH")
                nc.vector.tensor_scalar_mul(AH[:], psb[:], RS2[:, m, :])
                nc.vector.tensor_tensor(AH[:], AH[:], BB2[:], op=ALU.add)
                nc.vector.tensor_scalar_max(AH[:], AH[:], 0.0)
                XS = pd2.tile([64, H], f32r, name="XS")
                nc.vector.tensor_tensor(XS[:], AH[:], HmM[:, m, :], op=ALU.add)
                psm = pdps.tile([1, H], f32, name="ps")
                nc.tensor.matmul(psm[:], OnesC[0:64, :], XS[:],
                                 start=True, stop=True)
                nc.vector.tensor_scalar_mul(OutS[:, m, :], psm[:], 1.0 / APM)
            nc.sync.dma_start(out_mol[:].rearrange("(o m) h -> o m h", o=1), OutS[:])
        nc.leave_named_scope("D_readout", ns_D, False)

    nc.compile()
    return nc


def _host_prepare(inputs):
    f_atoms = np.asarray(inputs["f_atoms"], np.float32)
    f_bonds = np.asarray(inputs["f_bonds"], np.float32)
    a2b = np.asarray(inputs["a2b"]).astype(np.int64)
    b2a = np.asarray(inputs["b2a"]).astype(np.int64)
    b2revb = np.asarray(inputs["b2revb"]).astype(np.int64)

    fbp = np.zeros((B, H), np.float32)
    fbp[:, :FD] = f_bonds
    fbT_full = np.ascontiguousarray(fbp.T)
    faT_full = np.ascontiguousarray(f_atoms.T)

    W = {k: np.asarray(inputs[k], np.float32) for k in
         ("Wq_atom", "Wk_atom", "Wv_atom", "Wi", "Wh", "Wq", "Wk", "Wv", "Wa",
          "Walpha_w", "Wo_w", "Wb_w")}
    wi_p = np.zeros((H, H), np.float32)
    wi_p[:FD, :] = W["Wi"]
    wqk = np.ascontiguousarray(W["Wq"] @ W["Wk"].T)
    wva = np.ascontiguousarray(W["Wv"] @ W["Wa"])

    base = dict(
        wq_a=W["Wq_atom"], wk_a=W["Wk_atom"], wv_a=W["Wv_atom"],
        ln_g=np.asarray(inputs["ln_g"], np.float32).reshape(1, F),
        ln_b=np.asarray(inputs["ln_b"], np.float32).reshape(1, F),
        wi=wi_p, wqk=wqk, wh=W["Wh"], wva=wva, wa=W["Wa"],
        wal1=np.ascontiguousarray(W["Walpha_w"][:H]),
        wal2=np.ascontiguousarray(W["Walpha_w"][H:]),
        nwalb=-np.asarray(inputs["Walpha_b"], np.float32).reshape(1, 1),
        wo_f=np.ascontiguousarray(W["Wo_w"][:F]),
        wo_m=np.ascontiguousarray(W["Wo_w"][F:]),
        wo_b=np.asarray(inputs["Wo_b"], np.float32).reshape(1, H),
        wb_w=W["Wb_w"],
        wb_b=np.asarray(inputs["Wb_b"], np.float32).reshape(1, H),
        ident=np.eye(128, dtype=np.float32),
        onesr=np.ones((1, 128), np.float32),
        onesc=np.ones((128, 1), np.float32),
    )

    in_maps = []
    for c in range(NC):
        bonds = np.arange(c * BS, (c + 1) * BS)
        # AG payload row of global bond b: rank block of 2048 rows, msgN first
        grow = lambda b: (b // BS) * PAYR + (b % BS)
        # [7, BS]: rows 0..5 = a2b[b2a] terms (added), row 6 = b2revb (subbed)
        terms = np.stack([grow(a2b[b2a[bonds], j]) for j in range(MNB)]
                         + [grow(b2revb[bonds])])
        nb2 = np.ascontiguousarray(
            terms.reshape(7, BS // 128, 128).transpose(2, 0, 1)).astype(np.int32)
        atoms = np.arange(c * AS, (c + 1) * AS)
        ro = np.stack([a2b[atoms, j] for j in range(MNB)])  # [6, AS]
        ro2 = np.ascontiguousarray(
            ro.reshape(MNB, AS // 128, 128).transpose(2, 0, 1)).astype(np.int32)
        m = dict(base)
        m["fbT"] = np.ascontiguousarray(fbT_full[:, bonds])
        m["faT"] = np.ascontiguousarray(faT_full[:, atoms])
        m["fa"] = np.ascontiguousarray(f_atoms[atoms])
        m["nb_idx"] = nb2
        m["ro_idx"] = ro2
        in_maps.append(m)
    return in_maps


def kernel(**inputs):
    if "nc" not in _CACHE:
        _CACHE["nc"] = _build()
    nc = _CACHE["nc"]
    in_maps = _host_prepare(inputs)
    res = run_bass_kernel_spmd(nc, in_maps, core_ids=list(range(NC)))
    out = np.concatenate([res.results[c]["mol_out"] for c in range(NC)], 0)
    return np.ascontiguousarray(out.astype(np.float32))


# revision 45
# speedup vs baseline: 1.3101x; 1.0208x over previous
"""Trainium2 Bass kernel for nn_HGNNEncoder (DMPNN + global bond attention).

Sharding: data-parallel over bonds/atoms/molecules across 8 NeuronCores.

Key structure (v2):
 - Weight folding: scores = (M Wq)(M Wk)^T = M Wqk M^T with Wqk = Wq Wk^T,
   and att = (P V) Wa = (P M)(Wv Wa) = (P M) Wva.  So the attention needs
   only the raw message M as both K and V -> no K/V compute at all.
 - One AllGather per iteration boundary carrying [msgN | msgT] in bf16
   (1 MB per rank).  M^T (for QK lhsT) and M-natural+ones (for PV rhs)
   are DMA'd back from the AG output.
 - DMPNN neighbor-sum via one dma_gather(transpose=True) per iteration
   (7*1024 rows, bf16) + DVE tree-sum -> directly transposed T^T.
 - bf16 everywhere on the matmul paths, fp32 PSUM accumulation, exp on
   ScalarE with the 1/16 scale folded in, sigmoid computed as
   1/(1+exp(-x)) to avoid ACT table swaps.
"""

import numpy as np

import concourse.bass as bass
import concourse.bacc as bacc
import concourse.mybir as mybir
import concourse.tile as tile
from concourse.bass_utils import run_bass_kernel_spmd

NC = 8
B, NA, MNB = 8192, 4096, 6
H = 256
F = 133
FD = 147
BS = B // NC          # 1024 bonds per core
AS = NA // NC         # 512 atoms per core
APM = 64              # atoms per molecule
MPC = AS // APM       # 8 molecules per core
NITER = 3
NBI = 7 * BS          # 7168 dmpnn gather idxs
ROI = MNB * AS        # 3072 readout gather idxs
PAYR = 2 * BS         # 2048 rows of 256 per rank in the AG payload

f32 = mybir.dt.float32
f32r = mybir.dt.float32r
bf16 = mybir.dt.bfloat16
i16 = mybir.dt.int16
AF = mybir.ActivationFunctionType
ALU = mybir.AluOpType
AX = mybir.AxisListType

SCALE_BOND = float(1.0 / np.sqrt(np.float32(H)))
SCALE_ATOM = float(1.0 / np.sqrt(np.float32(F)))

_CACHE = {}


def _build():
    nc = bacc.Bacc("TRN2", target_bir_lowering=False, debug=False, num_devices=NC)

    def inp(name, shape, dt=f32):
        return nc.dram_tensor(name, list(shape), dt, kind="ExternalInput")

    fbT = inp("fbT", [H, BS])
    faT = inp("faT", [F, AS])
    fa = inp("fa", [AS, F])
    wq_a = inp("wq_a", [F, F]); wk_a = inp("wk_a", [F, F]); wv_a = inp("wv_a", [F, F])
    ln_g = inp("ln_g", [1, F]); ln_b = inp("ln_b", [1, F])
    wi = inp("wi", [H, H])
    wqk = inp("wqk", [H, H]); wh = inp("wh", [H, H]); wva = inp("wva", [H, H])
    wa = inp("wa", [H, H])
    wal1 = inp("wal1", [H, 1]); wal2 = inp("wal2", [H, 1]); nwalb = inp("nwalb", [1, 1])
    wo_f = inp("wo_f", [F, H]); wo_m = inp("wo_m", [H, H]); wo_b = inp("wo_b", [1, H])
    wb_w = inp("wb_w", [H, H]); wb_b = inp("wb_b", [1, H])
    ident_in = inp("ident", [128, 128])
    onesr_in = inp("onesr", [1, 128])
    onesc_in = inp("onesc", [128, 1])
    nb_idx = inp("nb_idx", [128, 7, BS // 128], mybir.dt.int32)
    ro_idx = inp("ro_idx", [128, MNB, AS // 128], mybir.dt.int32)
    out_mol = nc.dram_tensor("mol_out", [MPC, H], f32, kind="ExternalOutput")

    with tile.TileContext(nc) as tc, \
         tc.tile_pool(name="persist", bufs=1) as per, \
         tc.tile_pool(name="dram", bufs=1, space="DRAM") as dram, \
         tc.tile_pool(name="psA", bufs=2, space="PSUM") as ppsA:

        def load_round(pool, src_ap, shape, name, dt=f32r, raw_pool=None):
            raw = (raw_pool or pool).tile(list(shape), f32, name=name + "_raw")
            nc.sync.dma_start(raw[:], src_ap)
            if dt == f32:
                return raw
            t = pool.tile(list(shape), dt, name=name)
            nc.vector.tensor_copy(t[:], raw[:])
            return t

        # ---------------- persistent weights ----------------
        with tc.tile_pool(name="raws", bufs=1) as raws:
            def loadw(src, name, cols=H, dt=bf16):
                return load_round(per, src[:].rearrange("(t p) h -> p t h", p=128),
                                  [128, 2, cols], name, dt=dt, raw_pool=raws)
            Wi = loadw(wi, "Wi", dt=f32r); Wqk = loadw(wqk, "Wqk", dt=f32r)
            Wh = loadw(wh, "Wh", dt=f32r)
            Wva = loadw(wva, "Wva"); Wom = loadw(wo_m, "Wom", dt=f32r)
            War = loadw(wa, "War", dt=f32r); Wbw = loadw(wb_w, "Wbw", dt=f32r)
            Wal1 = loadw(wal1, "Wal1", cols=1, dt=f32r)
            Wal2 = loadw(wal2, "Wal2", cols=1, dt=f32r)
            Wof_h = load_round(per, wo_f[0:128, :], [128, H], "Wof_h", raw_pool=raws)
            Wof_l = load_round(per, wo_f[128:F, :], [5, H], "Wof_l", raw_pool=raws)
            Wob = load_round(per, wo_b[:], [1, H], "Wob", raw_pool=raws)
            Wbb = load_round(per, wb_b[:], [1, H], "Wbb", raw_pool=raws)
            Ident = load_round(per, ident_in[:], [128, 128], "Ident", dt=f32)
            IdentB = load_round(per, ident_in[:], [128, 128], "IdentB", dt=bf16,
                                raw_pool=raws)
            Onesr = load_round(per, onesr_in[:], [1, 128], "Onesr", raw_pool=raws)
            OnesC = load_round(per, onesc_in[:], [128, 1], "OnesC", raw_pool=raws)
            NWalb = load_round(per, nwalb[:], [1, 1], "NWalb", dt=f32)
            OnesrF = load_round(per, onesr_in[:], [1, 128], "OnesrF", dt=f32)
            NbIdx = per.tile([128, 7, BS // 128], mybir.dt.int32, name="NbIdx")
            nc.sync.dma_start(NbIdx[:], nb_idx[:])
            RoIdx = per.tile([128, MNB, AS // 128], mybir.dt.int32, name="RoIdx")
            nc.sync.dma_start(RoIdx[:], ro_idx[:])

        # persistent activations
        InpT = per.tile([128, 2, BS], f32, name="InpT")
        MsgTx = per.tile([128, 2, BS], bf16, name="MsgTx")
        MsgTxF = per.tile([128, 2, BS], f32r, name="MsgTxF")
        MsgN = per.tile([128, BS // 128, H], bf16, name="MsgN")
        FeT_h = per.tile([128, AS], f32r, name="FeT_h")
        FeT_l = per.tile([5, AS], f32r, name="FeT_l")

        agx_in = [dram.tile([PAYR, H], bf16, name=f"agx_in{t}") for t in range(NITER)]
        agx_out = [dram.tile([NC * PAYR, H], bf16, name=f"agx_out{t}",
                             addr_space="Shared") for t in range(NITER)]
        agn_in = dram.tile([BS, H], bf16, name="agn_in")
        agn_out = dram.tile([B, H], bf16, name="agn_out", addr_space="Shared")
        RG = [list(range(NC))]

        def boundary(it):
            # natural-layout local message via PE transposes
            for c in range(BS // 128):
                psn = ppsA.tile([128, 512], f32, name="psA")[:].bitcast(bf16)
                for ht in range(2):
                    nc.tensor.transpose(psn[:, bass.ts(ht, 128)],
                                        MsgTx[:, ht, bass.ts(c, 128)], IdentB[:])
                nc.vector.tensor_copy(MsgN[:, c, :], psn[:, 0:H])
            if it < NITER:
                nc.sync.dma_start(
                    agx_in[it][0:BS, :].rearrange("(lb p) e -> p lb e", p=128),
                    MsgN[:])
                nc.sync.dma_start(
                    agx_in[it][BS:PAYR, :].rearrange("(t p jj) e -> p t (jj e)",
                                                     t=2, p=128),
                    MsgTx[:])
                nc.gpsimd.collective_compute(
                    "AllGather", ALU.bypass, RG,
                    ins=[agx_in[it].opt()], outs=[agx_out[it].opt()])
            else:
                nc.sync.dma_start(
                    agn_in[:].rearrange("(lb p) e -> p lb e", p=128), MsgN[:])
                nc.gpsimd.collective_compute(
                    "AllGather", ALU.bypass, RG,
                    ins=[agn_in.opt()], outs=[agn_out.opt()])

        def qt_prime():
            # Q'^T = Wqk^T @ M^T  (local shard, f32r for precision)
            for ht in range(2):
                for ch in range(2):
                    ps = ppsA.tile([128, 512], f32, name="psA")
                    for kt in range(2):
                        nc.tensor.matmul(
                            ps[:], Wqk[:, kt, bass.ts(ht, 128)],
                            MsgTxF[:, kt, bass.ts(ch, 512)],
                            start=(kt == 0), stop=(kt == 1))
                    nc.vector.tensor_copy(QT[:, ht, bass.ts(ch, 512)], ps[:])

        # ============ Phase B: message_0 = relu(f_bonds @ Wi) ==============
        ns_B = nc.enter_named_scope("B_init", False)[0]
        with tc.tile_pool(name="pb", bufs=1) as pb:
            FbT = load_round(pb, fbT[:].rearrange("(t p) i -> p t i", p=128),
                             [128, 2, BS], "FbT")
            for ht in range(2):
                for ch in range(2):
                    ps = ppsA.tile([128, 512], f32, name="psA")
                    for kt in range(2):
                        nc.tensor.matmul(ps[:], Wi[:, kt, bass.ts(ht, 128)],
                                         FbT[:, kt, bass.ts(ch, 512)],
                                         start=(kt == 0), stop=(kt == 1))
                    nc.vector.tensor_copy(InpT[:, ht, bass.ts(ch, 512)], ps[:])
                    nc.scalar.activation(MsgTx[:, ht, bass.ts(ch, 512)], ps[:],
                                         AF.Relu)
                    nc.scalar.activation(MsgTxF[:, ht, bass.ts(ch, 512)], ps[:],
                                         AF.Relu)
            boundary(0)
        nc.leave_named_scope("B_init", ns_B, False)

        # ============ Phase C: 3 message-passing iterations ================
        with tc.tile_pool(name="psS", bufs=2, space="PSUM") as ppsS, \
             tc.tile_pool(name="psP", bufs=1, space="PSUM") as ppsP, \
             tc.tile_pool(name="pmt", bufs=2) as pmt, \
             tc.tile_pool(name="pcw", bufs=1) as pcw:
            QT = pcw.tile([128, 2, BS], bf16, name="QT")
            MT = pcw.tile([128, 2, NC, BS], bf16, name="MT")
            Mb = pcw.tile([128, B // 128, H + 1], bf16, name="Mb")
            Gn = pcw.tile([128, 7, BS // 128, H], bf16, name="Gn")
            T1 = pcw.tile([128, BS // 128, H], f32, name="T1")
            TT = pcw.tile([128, 2, BS], f32r, name="TT")
            DmT = pcw.tile([128, 2, BS], f32r, name="DmT")
            AmT = pcw.tile([128, 2, BS], f32r, name="AmT")
            AtT = pcw.tile([128, 2, BS], bf16, name="AtT")
            AttA = pcw.tile([128, BS // 128, H], bf16, name="AttA")
            AlB = pcw.tile([128, BS], f32, name="AlB")
            Alp = pcw.tile([1, BS], f32r, name="Alp")
            Ea = pcw.tile([1, BS], f32, name="Ea")
            D1 = pcw.tile([128, BS], f32, name="D1")
            nc.vector.memset(Mb[:], 1.0)   # ones column persists across iters
            qt_prime()
            for it in range(NITER):
                ns_it = nc.enter_named_scope(f"C{it}", False)[0]
                src = agx_out[it]
                # ---- DMA-in M^T and M-natural(+ones) from the AG output
                for c in range(NC):
                    nc.sync.dma_start(
                        MT[:, :, c, :],
                        src[c * PAYR + BS:(c + 1) * PAYR, :]
                        .rearrange("(t p jj) e -> p t (jj e)", t=2, p=128))
                for c in range(NC):
                    nc.sync.dma_start(
                        Mb[:, c * 8:(c + 1) * 8, 0:H],
                        src[c * PAYR:c * PAYR + BS, :]
                        .rearrange("(lb p) e -> p lb e", p=128))
                # ---- dmpnn gather (batched rows); sums/transposes after attn
                ns_g = nc.enter_named_scope(f"C{it}_gather", False)[0]
                for k in range(7):
                    for c in range(BS // 128):
                        nc.gpsimd.indirect_dma_start(
                            out=Gn[:, k, c, :], out_offset=None, in_=src[:],
                            in_offset=bass.IndirectOffsetOnAxis(
                                ap=NbIdx[:, k, c:c + 1], axis=0))
                nc.leave_named_scope(f"C{it}_gather", ns_g, False)

                def att_half(half):
                    # att^T and att_msg^T for one 512-bond half
                    for ht in range(2):
                        pst = ppsA.tile([128, 512], f32,
                                        name="psA")[:].bitcast(bf16)
                        for cc in range(4):
                            nc.tensor.transpose(
                                pst[:, bass.ts(cc, 128)],
                                AttA[:, half * 4 + cc, bass.ts(ht, 128)],
                                IdentB[:])
                        nc.vector.tensor_copy(AtT[:, ht, bass.ts(half, 512)],
                                              pst[:, 0:512])
                    for ht in range(2):
                        ps = ppsA.tile([128, 512], f32, name="psA")
                        for kt in range(2):
                            nc.tensor.matmul(ps[:], Wva[:, kt, bass.ts(ht, 128)],
                                             AtT[:, kt, bass.ts(half, 512)],
                                             start=(kt == 0), stop=(kt == 1))
                        nc.vector.tensor_copy(AmT[:, ht, bass.ts(half, 512)],
                                              ps[:])

                # ---- attention (rows = shard), flash-style over j blocks
                ns_at = nc.enter_named_scope(f"C{it}_attn", False)[0]
                for ic in range(2):
                    pvp = ppsP.tile([128, 4, 512], f32, name="psP")
                    prev = None
                    for jj in range(B // 128 + 1):
                        if jj < B // 128:
                            c, jl = divmod(jj, 8)
                            sp = ppsS.tile([128, 512], f32, name="psS")
                            for kt in range(2):
                                nc.tensor.matmul(
                                    sp[:], MT[:, kt, c, bass.ts(jl, 128)],
                                    QT[:, kt, bass.ts(ic, 512)],
                                    start=(kt == 0), stop=(kt == 1))
                            pt = pmt.tile([128, 512], bf16, name="PT")
                            nc.scalar.activation(pt[:], sp[:], AF.Exp,
                                                 scale=SCALE_BOND)
                        if prev is not None:
                            pj, ppt = prev
                            for isub in range(4):
                                nc.tensor.matmul(
                                    pvp[:, isub, 0:H + 1],
                                    ppt[:, bass.ts(isub, 128)],
                                    Mb[:, pj, 0:H + 1],
                                    start=(pj == 0), stop=(pj == B // 128 - 1))
                        if jj < B // 128:
                            prev = (jj, pt)
                    for isub in range(4):
                        rec = pmt.tile([128, 1], f32, name="rec")
                        nc.vector.reciprocal(rec[:], pvp[:, isub, H:H + 1])
                        nc.vector.tensor_scalar_mul(
                            AttA[:, ic * 4 + isub, :], pvp[:, isub, 0:H], rec[:])
                    if ic == 0:
                        # overlap dmpnn tail + first att half with ic=1's MMs
                        g = lambda k: Gn[:, k, :, :]
                        # tree-sum scratch aliases AmT (written later below)
                        T2v = AmT[:].rearrange("p t (c e) -> p (t c) e", e=256)
                        nc.vector.tensor_tensor(T1[:], g(0), g(1), op=ALU.add)
                        nc.vector.tensor_tensor(T2v, g(2), g(3), op=ALU.add)
                        nc.vector.tensor_tensor(T1[:], T1[:], T2v, op=ALU.add)
                        nc.vector.tensor_tensor(T2v, g(4), g(5), op=ALU.add)
                        nc.vector.tensor_tensor(T1[:], T1[:], T2v, op=ALU.add)
                        nc.vector.tensor_scalar_mul(T2v, g(6), -1.0)
                        nc.vector.tensor_tensor(T1[:], T1[:], T2v, op=ALU.add)
                        for c in range(BS // 128):
                            pst = ppsA.tile([128, 512], f32, name="psA")
                            for kt in range(2):
                                nc.tensor.transpose(pst[:, bass.ts(kt, 128)],
                                                    T1[:, c, bass.ts(kt, 128)],
                                                    Ident[:])
                            nc.vector.tensor_copy(
                                TT[:, :, c * 128:(c + 1) * 128],
                                pst[:, 0:256].rearrange("p (t x) -> p t x", t=2))
                        for ht in range(2):
                            for ch in range(2):
                                ps = ppsA.tile([128, 512], f32, name="psA")
                                for kt in range(2):
                                    nc.tensor.matmul(
                                        ps[:], Wh[:, kt, bass.ts(ht, 128)],
                                        TT[:, kt, bass.ts(ch, 512)],
                                        start=(kt == 0), stop=(kt == 1))
                                nc.vector.tensor_copy(
                                    DmT[:, ht, bass.ts(ch, 512)], ps[:])
                        att_half(0)
                nc.leave_named_scope(f"C{it}_attn", ns_at, False)
                ns_po = nc.enter_named_scope(f"C{it}_post", False)[0]
                att_half(1)

                # ---- alpha = sigmoid(w1.dm + w2.am + b) via exp on ScalarE
                for ch in range(2):
                    ps = ppsA.tile([1, 512], f32, name="psA")
                    nc.tensor.matmul(ps[:], Wal1[:, 0, :],
                                     DmT[:, 0, bass.ts(ch, 512)],
                                     start=True, stop=False)
                    nc.tensor.matmul(ps[:], Wal1[:, 1, :],
                                     DmT[:, 1, bass.ts(ch, 512)],
                                     start=False, stop=False)
                    nc.tensor.matmul(ps[:], Wal2[:, 0, :],
                                     AmT[:, 0, bass.ts(ch, 512)],
                                     start=False, stop=False)
                    nc.tensor.matmul(ps[:], Wal2[:, 1, :],
                                     AmT[:, 1, bass.ts(ch, 512)],
                                     start=False, stop=True)
                    nc.scalar.activation(Ea[:, bass.ts(ch, 512)], ps[:],
                                         AF.Exp, scale=-1.0, bias=NWalb[:])
                nc.vector.tensor_scalar_add(Ea[:], Ea[:], 1.0)
                with nc.allow_low_precision(reason="f32r alpha"):
                    nc.vector.reciprocal(Alp[:], Ea[:])
                for ch in range(2):
                    ps = ppsA.tile([128, 512], f32, name="psA")
                    nc.tensor.matmul(ps[:], Onesr[:],
                                     Alp[:, bass.ts(ch, 512)],
                                     start=True, stop=True)
                    nc.vector.tensor_copy(AlB[:, bass.ts(ch, 512)], ps[:])

                # ---- combine (f32)
                for ht in range(2):
                    nc.vector.tensor_tensor(D1[:], DmT[:, ht, :],
                                            AmT[:, ht, :], op=ALU.subtract)
                    nc.vector.tensor_tensor(D1[:], D1[:], AlB[:], op=ALU.mult)
                    nc.vector.tensor_tensor(D1[:], D1[:], AmT[:, ht, :],
                                            op=ALU.add)
                    nc.vector.tensor_tensor(D1[:], D1[:], InpT[:, ht, :],
                                            op=ALU.add)
                    nc.scalar.activation(MsgTx[:, ht, :], D1[:], AF.Relu)
                    nc.scalar.activation(MsgTxF[:, ht, :], D1[:], AF.Relu)
                boundary(it + 1)
                if it + 1 < NITER:
                    qt_prime()
                nc.leave_named_scope(f"C{it}_post", ns_po, False)
                nc.leave_named_scope(f"C{it}", ns_it, False)

        # ============ D part 1: a_message gather (independent of A) ========
        ns_Dg = nc.enter_named_scope("D_gather", False)[0]
        pdg = tc.alloc_tile_pool(name="pdg", bufs=1)
        Gr = pdg.tile([128, MNB, AS // 128, H], bf16, name="Gr")
        for k in range(MNB):
            for c in range(AS // 128):
                nc.gpsimd.indirect_dma_start(
                    out=Gr[:, k, c, :], out_offset=None,
                    in_=agn_out[:],
                    in_offset=bass.IndirectOffsetOnAxis(
                        ap=RoIdx[:, k, c:c + 1], axis=0))
        A1 = pdg.tile([128, AS // 128, H], f32, name="A1")
        A2 = pdg.tile([128, AS // 128, H], f32, name="A2")
        gr = lambda k: Gr[:, k, :, :]
        nc.vector.tensor_tensor(A1[:], gr(0), gr(1), op=ALU.add)
        nc.vector.tensor_tensor(A2[:], gr(2), gr(3), op=ALU.add)
        nc.vector.tensor_tensor(A1[:], A1[:], A2[:], op=ALU.add)
        nc.vector.tensor_tensor(A2[:], gr(4), gr(5), op=ALU.add)
        nc.vector.tensor_tensor(A1[:], A1[:], A2[:], op=ALU.add)
        AmT2 = pdg.tile([128, 2, AS], f32r, name="AmT2")
        for c in range(AS // 128):
            pst = ppsA.tile([128, 512], f32, name="psA")
            for kt in range(2):
                nc.tensor.transpose(pst[:, bass.ts(kt, 128)],
                                    A1[:, c, bass.ts(kt, 128)], Ident[:])
            nc.vector.tensor_copy(
                AmT2[:, :, c * 128:(c + 1) * 128],
                pst[:, 0:256].rearrange("p (t x) -> p t x", t=2))
        nc.leave_named_scope("D_gather", ns_Dg, False)

        # ============ Phase A: per-molecule atom self-attention ============
        # (emitted late: fills the final-AG wait; only feeds the readout)
        ns_A = nc.enter_named_scope("A_atoms", False)[0]
        with tc.tile_pool(name="pa", bufs=1) as pa, \
             tc.tile_pool(name="pa2", bufs=2) as pa2, \
             tc.tile_pool(name="pa_ps", bufs=4, space="PSUM") as paps:
            WqA_h = load_round(pa, wq_a[0:128, :], [128, F], "WqA_h", dt=f32)
            WqA_l = load_round(pa, wq_a[128:F, :], [5, F], "WqA_l", dt=f32)
            WkA_h = load_round(pa, wk_a[0:128, :], [128, F], "WkA_h", dt=f32)
            WkA_l = load_round(pa, wk_a[128:F, :], [5, F], "WkA_l", dt=f32)
            WvA_h = load_round(pa, wv_a[0:128, :], [128, F], "WvA_h", dt=f32)
            WvA_l = load_round(pa, wv_a[128:F, :], [5, F], "WvA_l", dt=f32)
            LnG = load_round(pa, ln_g[:], [1, F], "LnG", dt=f32)
            LnB = load_round(pa, ln_b[:], [1, F], "LnB", dt=f32)
            FaT_h = load_round(pa, faT[0:128, :], [128, AS], "FaT_h", dt=f32)
            FaT_l = load_round(pa, faT[128:F, :], [5, AS], "FaT_l", dt=f32)
            XN = pa.tile([64, MPC, F], f32, name="XN")
            nc.sync.dma_start(XN[:], fa[:].rearrange("(m a) f -> a m f", a=64))
            GB = pa.tile([64, F], f32, name="GB")
            BB = pa.tile([64, F], f32, name="BB")
            for bc_src, bc_dst in ((LnG, GB), (LnB, BB)):
                ps = paps.tile([64, F], f32, name="ps")
                nc.tensor.matmul(ps[:], OnesrF[:, 0:64], bc_src[:],
                                 start=True, stop=True)
                nc.vector.tensor_copy(bc_dst[:], ps[:])

            def mm133(dst, lhs_pair, rhs_pair, n):
                ps = paps.tile([dst.shape[0], n], f32, name="ps")
                nc.tensor.matmul(ps[:], lhs_pair[0], rhs_pair[0],
                                 start=True, stop=False)
                nc.tensor.matmul(ps[:], lhs_pair[1], rhs_pair[1],
                                 start=False, stop=True)
                nc.vector.tensor_copy(dst, ps[:])

            QTa_h = pa.tile([128, AS], f32, name="QTa_h")
            QTa_l = pa.tile([5, AS], f32, name="QTa_l")
            KTa_h = pa.tile([128, AS], f32, name="KTa_h")
            KTa_l = pa.tile([5, AS], f32, name="KTa_l")
            mm133(QTa_h[:], (WqA_h[:, 0:128], WqA_l[:, 0:128]),
                  (FaT_h[:], FaT_l[:]), AS)
            mm133(QTa_l[:], (WqA_h[:, 128:F], WqA_l[:, 128:F]),
                  (FaT_h[:], FaT_l[:]), AS)
            mm133(KTa_h[:], (WkA_h[:, 0:128], WkA_l[:, 0:128]),
                  (FaT_h[:], FaT_l[:]), AS)
            mm133(KTa_l[:], (WkA_h[:, 128:F], WkA_l[:, 128:F]),
                  (FaT_h[:], FaT_l[:]), AS)
            VN = pa.tile([64, MPC, F], f32, name="VN")
            for m in range(MPC):
                mm133(VN[:, m, :],
                      (FaT_h[:, bass.ts(m, 64)], FaT_l[:, bass.ts(m, 64)]),
                      (WvA_h[:], WvA_l[:]), F)
            E = pa.tile([64, MPC, 64], f32, name="E")
            for m in range(MPC):
                ps = paps.tile([64, 64], f32, name="ps")
                nc.tensor.matmul(ps[:], QTa_h[:, bass.ts(m, 64)],
                                 KTa_h[:, bass.ts(m, 64)], start=True, stop=False)
                nc.tensor.matmul(ps[:], QTa_l[:, bass.ts(m, 64)],
                                 KTa_l[:, bass.ts(m, 64)], start=False, stop=True)
                nc.scalar.activation(E[:, m, :], ps[:], AF.Exp, scale=SCALE_ATOM)
            SumsA = pa.tile([64, MPC, 1], f32, name="SumsA")
            RS = pa.tile([64, MPC, 1], f32, name="RS")
            nc.vector.tensor_reduce(SumsA[:], E[:], axis=AX.X, op=ALU.add)
            nc.vector.reciprocal(RS[:], SumsA[:])
            AttnN = pa.tile([64, MPC, F], f32, name="AttnN")
            for m in range(MPC):
                pst = paps.tile([64, 64], f32, name="ps")
                nc.tensor.transpose(pst[:], E[:, m, :], Ident[0:64, 0:64])
                ET = pa2.tile([64, 64], f32, name="ET")
                nc.vector.tensor_copy(ET[:], pst[:])
                ps = paps.tile([64, F], f32, name="ps")
                nc.tensor.matmul(ps[:], ET[:], VN[:, m, :], start=True, stop=True)
                nc.vector.tensor_scalar_mul(AttnN[:, m, :], ps[:], RS[:, m, :])
            SumX = pa.tile([64, MPC, F], f32, name="SumX")
            nc.vector.tensor_tensor(SumX[:], XN[:], AttnN[:], op=ALU.add)
            Mu = pa.tile([64, MPC, 1], f32, name="Mu")
            nc.vector.tensor_reduce(Mu[:], SumX[:], axis=AX.X, op=ALU.add)
            nc.vector.tensor_scalar_mul(Mu[:], Mu[:], 1.0 / F)
            XC = pa.tile([64, MPC, F], f32, name="XC")
            nc.vector.tensor_tensor(XC[:], SumX[:],
                                    Mu[:].to_broadcast([64, MPC, F]),
                                    op=ALU.subtract)
            SQ = pa.tile([64, MPC, F], f32, name="SQ")
            nc.vector.tensor_tensor(SQ[:], XC[:], XC[:], op=ALU.mult)
            Var = pa.tile([64, MPC, 1], f32, name="Var")
            nc.vector.tensor_reduce(Var[:], SQ[:], axis=AX.X, op=ALU.add)
            Std = pa.tile([64, MPC, 1], f32, name="Std")
            EpsT = pa.tile([64, 1], f32, name="EpsT")
            nc.vector.memset(EpsT[:], 1e-5)
            nc.scalar.activation(Std[:], Var[:], AF.Sqrt, scale=1.0 / F,
                                 bias=EpsT[:])
            RStd = pa.tile([64, MPC, 1], f32, name="RStd")
            nc.vector.reciprocal(RStd[:], Std[:])
            FeN = pa.tile([64, MPC, F], f32, name="FeN")
            nc.vector.tensor_tensor(XC[:], XC[:],
                                    RStd[:].to_broadcast([64, MPC, F]),
                                    op=ALU.mult)
            nc.vector.tensor_tensor(XC[:], XC[:],
                                    GB[:, None, :].to_broadcast([64, MPC, F]),
                                    op=ALU.mult)
            nc.vector.tensor_tensor(FeN[:], XC[:],
                                    BB[:, None, :].to_broadcast([64, MPC, F]),
                                    op=ALU.add)
            for m in range(MPC):
                ps1 = paps.tile([128, 64], f32, name="ps")
                nc.tensor.transpose(ps1[:], FeN[:, m, 0:128], Ident[0:64, 0:64])
                nc.vector.tensor_copy(FeT_h[:, bass.ts(m, 64)], ps1[:])
                ps2 = paps.tile([5, 64], f32, name="ps")
                nc.tensor.transpose(ps2[:], FeN[:, m, 128:F], Ident[0:64, 0:64])
                nc.vector.tensor_copy(FeT_l[:, bass.ts(m, 64)], ps2[:])
        nc.leave_named_scope("A_atoms", ns_A, False)

        # ============ Readout + per-molecule pooling =======================
        ns_D = nc.enter_named_scope("D_readout", False)[0]
        with tc.tile_pool(name="pd", bufs=1) as pd, \
             tc.tile_pool(name="pd2", bufs=2) as pd2, \
             tc.tile_pool(name="pd_ps", bufs=4, space="PSUM") as pdps:
            Hm = pd.tile([128, AS // 128, H], f32r, name="Hm")
            for c in range(AS // 128):
                ps = pdps.tile([128, H], f32, name="ps")
                nc.tensor.matmul(ps[:], FeT_h[:, bass.ts(c, 128)], Wof_h[:],
                                 start=True, stop=False)
                nc.tensor.matmul(ps[:], FeT_l[:, bass.ts(c, 128)], Wof_l[:],
                                 start=False, stop=False)
                for kt in range(2):
                    nc.tensor.matmul(ps[:], AmT2[:, kt, bass.ts(c, 128)],
                                     Wom[:, kt, :], start=False, stop=False)
                nc.tensor.matmul(ps[:], Onesr[:, 0:128], Wob[:],
                                 start=False, stop=True)
                nc.scalar.activation(Hm[:, c, :], ps[:], AF.Relu)
            HmT = pd.tile([128, 2, AS], f32r, name="HmT")
            for ht in range(2):
                pst = pdps.tile([128, AS], f32, name="ps")
                for c in range(AS // 128):
                    nc.tensor.transpose(pst[:, bass.ts(c, 128)],
                                        Hm[:, c, bass.ts(ht, 128)].bitcast(f32),
                                        Ident[:])
                nc.vector.tensor_copy(HmT[:, ht, :], pst[:])
            # hm in per-molecule base-0 layout via SBUF->SBUF DMA
            HmM = pd.tile([64, MPC, H], f32r, name="HmM")
            nc.sync.dma_start(HmM[:, 0:MPC:2, :], Hm[0:64, :, :])
            nc.sync.dma_start(HmM[:, 1:MPC:2, :], Hm[64:128, :, :])
            T2T = pd.tile([128, 2, AS], f32r, name="T2T")
            for ht in range(2):
                ps = pdps.tile([128, AS], f32, name="ps")
                for kt in range(2):
                    nc.tensor.matmul(ps[:], War[:, kt, bass.ts(ht, 128)],
                                     HmT[:, kt, :], start=(kt == 0), stop=(kt == 1))
                nc.vector.tensor_copy(T2T[:, ht, :], ps[:])
            SC2 = pd.tile([64, MPC, 64], f32, name="SC2")
            for m in range(MPC):
                ps = pdps.tile([64, 64], f32, name="ps")
                for kt in range(2):
                    nc.tensor.matmul(ps[:], T2T[:, kt, bass.ts(m, 64)],
                                     HmT[:, kt, bass.ts(m, 64)],
                                     start=(kt == 0), stop=(kt == 1))
                nc.vector.tensor_copy(SC2[:, m, :], ps[:])
            Mx2 = pd.tile([64, MPC, 1], f32, name="Mx2")
            nc.vector.tensor_reduce(Mx2[:], SC2[:], axis=AX.X, op=ALU.max)
            NMx2 = pd.tile([64, MPC, 1], f32, name="NMx2")
            nc.vector.tensor_scalar_mul(NMx2[:], Mx2[:], -1.0)
            E2 = pd.tile([64, MPC, 64], f32, name="E2")
            for m in range(MPC):
                nc.scalar.activation(E2[:, m, :], SC2[:, m, :], AF.Exp,
                                     bias=NMx2[:, m, :])
            Sum2 = pd.tile([64, MPC, 1], f32, name="Sum2")
            RS2 = pd.tile([64, MPC, 1], f32, name="RS2")
            nc.vector.tensor_reduce(Sum2[:], E2[:], axis=AX.X, op=ALU.add)
            nc.vector.reciprocal(RS2[:], Sum2[:])
            BB2 = pd.tile([64, H], f32, name="BB2")
            psbb = pdps.tile([64, H], f32, name="ps")
            nc.tensor.matmul(psbb[:], Onesr[:, 0:64], Wbb[:], start=True, stop=True)
            nc.vector.tensor_copy(BB2[:], psbb[:])
            OutS = pd.tile([1, MPC, H], f32, name="OutS")
            for m in range(MPC):
                pst = pdps.tile([64, 64], f32, name="ps")
                nc.tensor.transpose(pst[:], E2[:, m, :], Ident[0:64, 0:64])
                E2T = pd2.tile([64, 64], f32r, name="E2T")
                nc.vector.tensor_copy(E2T[:], pst[:])
                UT = pd2.tile([128, 2, 64], f32r, name="UT")
                for hs in range(2):
                    psu = pdps.tile([128, 64], f32, name="ps")
                    nc.tensor.matmul(psu[:], HmM[:, m, bass.ts(hs, 128)], E2T[:],
                                     start=True, stop=True)
                    nc.vector.tensor_copy(UT[:, hs, :], psu[:])
                psb = pdps.tile([64, H], f32, name="ps")
                for kt in range(2):
                    nc.tensor.matmul(psb[:], UT[:, kt, :], Wbw[:, kt, :],
                                     start=(kt == 0), stop=(kt == 1))
                AH = pd2.tile([64, H], f32, name="AH")
                nc.vector.tensor_scalar_mul(AH[:], psb[:], RS2[:, m, :])
                nc.vector.tensor_tensor(AH[:], AH[:], BB2[:], op=ALU.add)
                nc.vector.tensor_scalar_max(AH[:], AH[:], 0.0)
                XS = pd2.tile([64, H], f32r, name="XS")
                nc.vector.tensor_tensor(XS[:], AH[:], HmM[:, m, :], op=ALU.add)
                psm = pdps.tile([1, H], f32, name="ps")
                nc.tensor.matmul(psm[:], OnesC[0:64, :], XS[:],
                                 start=True, stop=True)
                nc.vector.tensor_scalar_mul(OutS[:, m, :], psm[:], 1.0 / APM)
            nc.sync.dma_start(out_mol[:].rearrange("(o m) h -> o m h", o=1), OutS[:])
        nc.leave_named_scope("D_readout", ns_D, False)
        pdg.release()

    nc.compile()
    return nc


def _host_prepare(inputs):
    f_atoms = np.asarray(inputs["f_atoms"], np.float32)
    f_bonds = np.asarray(inputs["f_bonds"], np.float32)
    a2b = np.asarray(inputs["a2b"]).astype(np.int64)
    b2a = np.asarray(inputs["b2a"]).astype(np.int64)
    b2revb = np.asarray(inputs["b2revb"]).astype(np.int64)

    fbp = np.zeros((B, H), np.float32)
    fbp[:, :FD] = f_bonds
    fbT_full = np.ascontiguousarray(fbp.T)
    faT_full = np.ascontiguousarray(f_atoms.T)

    W = {k: np.asarray(inputs[k], np.float32) for k in
         ("Wq_atom", "Wk_atom", "Wv_atom", "Wi", "Wh", "Wq", "Wk", "Wv", "Wa",
          "Walpha_w", "Wo_w", "Wb_w")}
    wi_p = np.zeros((H, H), np.float32)
    wi_p[:FD, :] = W["Wi"]
    wqk = np.ascontiguousarray(W["Wq"] @ W["Wk"].T)
    wva = np.ascontiguousarray(W["Wv"] @ W["Wa"])

    base = dict(
        wq_a=W["Wq_atom"], wk_a=W["Wk_atom"], wv_a=W["Wv_atom"],
        ln_g=np.asarray(inputs["ln_g"], np.float32).reshape(1, F),
        ln_b=np.asarray(inputs["ln_b"], np.float32).reshape(1, F),
        wi=wi_p, wqk=wqk, wh=W["Wh"], wva=wva, wa=W["Wa"],
        wal1=np.ascontiguousarray(W["Walpha_w"][:H]),
        wal2=np.ascontiguousarray(W["Walpha_w"][H:]),
        nwalb=-np.asarray(inputs["Walpha_b"], np.float32).reshape(1, 1),
        wo_f=np.ascontiguousarray(W["Wo_w"][:F]),
        wo_m=np.ascontiguousarray(W["Wo_w"][F:]),
        wo_b=np.asarray(inputs["Wo_b"], np.float32).reshape(1, H),
        wb_w=W["Wb_w"],
        wb_b=np.asarray(inputs["Wb_b"], np.float32).reshape(1, H),
        ident=np.eye(128, dtype=np.float32),
        onesr=np.ones((1, 128), np.float32),
        onesc=np.ones((128, 1), np.float32),
    )

    in_maps = []
    for c in range(NC):
        bonds = np.arange(c * BS, (c + 1) * BS)
        # AG payload row of global bond b: rank block of 2048 rows, msgN first
        grow = lambda b: (b // BS) * PAYR + (b % BS)
        # [7, BS]: rows 0..5 = a2b[b2a] terms (added), row 6 = b2revb (subbed)
        terms = np.stack([grow(a2b[b2a[bonds], j]) for j in range(MNB)]
                         + [grow(b2revb[bonds])])
        nb2 = np.ascontiguousarray(
            terms.reshape(7, BS // 128, 128).transpose(2, 0, 1)).astype(np.int32)
        atoms = np.arange(c * AS, (c + 1) * AS)
        ro = np.stack([a2b[atoms, j] for j in range(MNB)])  # [6, AS]
        ro2 = np.ascontiguousarray(
            ro.reshape(MNB, AS // 128, 128).transpose(2, 0, 1)).astype(np.int32)
        m = dict(base)
        m["fbT"] = np.ascontiguousarray(fbT_full[:, bonds])
        m["faT"] = np.ascontiguousarray(faT_full[:, atoms])
        m["fa"] = np.ascontiguousarray(f_atoms[atoms])
        m["nb_idx"] = nb2
        m["ro_idx"] = ro2
        in_maps.append(m)
    return in_maps


def kernel(**inputs):
    if "nc" not in _CACHE:
        _CACHE["nc"] = _build()
    nc = _CACHE["nc"]
    in_maps = _host_prepare(inputs)
    res = run_bass_kernel_spmd(nc, in_maps, core_ids=list(range(NC)))
    out = np.concatenate([res.results[c]["mol_out"] for c in range(NC)], 0)
    return np.ascontiguousarray(out.astype(np.float32))
